# revision 3
# baseline (speedup 1.0000x reference)
import numpy as np
import ml_dtypes

import concourse.bacc as bacc
import concourse.bass as bass
import concourse.mybir as mybir
import concourse.tile as tile
from concourse import bass_utils

bf16 = ml_dtypes.bfloat16

B, N, D = 4, 2048, 1024
NQ, NK = 1024, 2048
FP32 = mybir.dt.float32
BF16 = mybir.dt.bfloat16
F32R = mybir.dt.float32r
EXP = mybir.ActivationFunctionType.Exp
SQRT = mybir.ActivationFunctionType.Sqrt

LAST_EXEC_NS = None
LAST_RESULT = None
_NC = None


def _broadcast_ap(dram_ap, parts):
    return bass.AP(
        tensor=dram_ap.tensor,
        offset=dram_ap.offset,
        ap=[[0, parts], dram_ap.ap[-1]],
    )


def _build():
    nc = bacc.Bacc(None, target_bir_lowering=False)
    qT = nc.dram_tensor("qT", [D, NQ], BF16, kind="ExternalInput")
    qn = nc.dram_tensor("qn", [NQ, D], FP32, kind="ExternalInput")
    kT = nc.dram_tensor("kT", [D, NK], BF16, kind="ExternalInput")
    vT = nc.dram_tensor("vT", [D, NK], BF16, kind="ExternalInput")
    wq = nc.dram_tensor("wq", [D, D], BF16, kind="ExternalInput")
    wk = nc.dram_tensor("wk", [D, D], BF16, kind="ExternalInput")
    wv = nc.dram_tensor("wv", [D, D], BF16, kind="ExternalInput")
    wo = nc.dram_tensor("wo", [D, D], BF16, kind="ExternalInput")
    gamma = nc.dram_tensor("gamma", [1, D], FP32, kind="ExternalInput")
    beta = nc.dram_tensor("beta", [1, D], FP32, kind="ExternalInput")
    out = nc.dram_tensor("out", [NQ, D], FP32, kind="ExternalOutput")

    with tile.TileContext(nc) as tc:
        with (
            tc.tile_pool(name="perm", bufs=1) as perm,
            tc.tile_pool(name="ps", bufs=1, space="PSUM") as ps,
        ):
            gamma_t = perm.tile([128, D], FP32)
            beta_t = perm.tile([128, D], FP32)
            nc.gpsimd.dma_start(out=gamma_t, in_=_broadcast_ap(gamma[0:1, :], 128))
            nc.gpsimd.dma_start(out=beta_t, in_=_broadcast_ap(beta[0:1, :], 128))
            eps_t = perm.tile([128, 1], FP32)
            nc.vector.memset(eps_t, 1e-5)
            ones_f = perm.tile([128, 64], FP32)
            nc.vector.memset(ones_f, 1.0)
            ones_t = perm.tile([128, 64], F32R)
            nc.vector.tensor_copy(ones_t, ones_f)

            wq_t = perm.tile([128, 8, D], BF16)
            wo_t = perm.tile([128, 8, D], BF16)
            nc.sync.dma_start(wq_t, wq[:, :].rearrange("(dt p) f -> p dt f", p=128))
            nc.sync.dma_start(wo_t, wo[:, :].rearrange("(dt p) f -> p dt f", p=128))

            # ksT[jt][p, m] = ks[m, jt*128+p]; vsp[kt][p, 65h:65h+64] = vs[kt*128+p, 64h:64h+64], col 65h+64 = 1
            ksT = [perm.tile([128, NK], BF16, tag=f"ks{j}", name=f"ks{j}") for j in range(8)]
            vsp = [perm.tile([128, 16 * 65], BF16, tag=f"vsp{t}", name=f"vsp{t}") for t in range(16)]

            with tc.tile_pool(name="kv", bufs=1) as kv:
                wkv_t = kv.tile([128, 8, D], BF16, tag="wkv")
                nc.sync.dma_start(wkv_t, wv[:, :].rearrange("(dt p) f -> p dt f", p=128))
                for rc in range(8):
                    vstage = kv.tile([128, 8, 256], BF16, tag="stage", bufs=2)
                    nc.sync.dma_start(
                        vstage,
                        vT[:, rc * 256:(rc + 1) * 256].rearrange("(dt p) n -> p dt n", p=128),
                    )
                    for rt in range(2):
                        kt_i = rc * 2 + rt
                        for jc in range(2):
                            pp = ps.tile([128, 512], FP32, tag="pp", bufs=2)
                            for dt_i in range(8):
                                nc.tensor.matmul(
                                    pp,
                                    vstage[:, dt_i, rt * 128:(rt + 1) * 128],
                                    wkv_t[:, dt_i, jc * 512:(jc + 1) * 512],
                                    start=(dt_i == 0),
                                    stop=(dt_i == 7),
                                )
                            nc.vector.tensor_copy(
                                vsp[kt_i].rearrange("p (h c) -> p h c", h=16)[:, jc * 8:(jc + 1) * 8, 0:64],
                                pp.rearrange("p (h c) -> p h c", h=8),
                            )
                        nc.vector.memset(
                            vsp[kt_i].rearrange("p (h c) -> p h c", h=16)[:, :, 64:65], 1.0
                        )

                nc.sync.dma_start(wkv_t, wk[:, :].rearrange("(dt p) f -> p dt f", p=128))
                for kc in range(4):
                    kstage = kv.tile([128, 8, 512], BF16, tag="kstage", bufs=2)
                    nc.sync.dma_start(
                        kstage,
                        kT[:, kc * 512:(kc + 1) * 512].rearrange("(dt p) n -> p dt n", p=128),
                    )
                    for jt in range(8):
                        pp = ps.tile([128, 512], FP32, tag="pp", bufs=2)
                        for dt_i in range(8):
                            nc.tensor.matmul(
                                pp,
                                wkv_t[:, dt_i, jt * 128:(jt + 1) * 128],
                                kstage[:, dt_i, :],
                                start=(dt_i == 0),
                                stop=(dt_i == 7),
                            )
                        nc.vector.tensor_copy(ksT[jt][:, kc * 512:(kc + 1) * 512], pp)

            with tc.tile_pool(name="qp", bufs=1) as qp:
                qstages = []
                for qc in range(2):
                    qst_ = qp.tile([128, 8, 512], BF16, tag="qstage", bufs=2,
                                   name=f"qstage{qc}")
                    nc.sync.dma_start(
                        qst_,
                        qT[:, qc * 512:(qc + 1) * 512].rearrange("(dt p) n -> p dt n", p=128),
                    )
                    qstages.append(qst_)
                for qc in range(2):
                    qstage = qstages[qc]
                    qs_t = [qp.tile([128, 512], BF16, tag=f"qs{j}", name=f"qs{j}") for j in range(8)]
                    for jt in range(8):
                        pp = ps.tile([128, 512], FP32, tag="pp", bufs=2)
                        for dt_i in range(8):
                            nc.tensor.matmul(
                                pp,
                                wq_t[:, dt_i, jt * 128:(jt + 1) * 128],
                                qstage[:, dt_i, :],
                                start=(dt_i == 0),
                                stop=(dt_i == 7),
                            )
                        nc.vector.tensor_copy(qs_t[jt], pp)

                    at_t = [qp.tile([128, 512], BF16, tag=f"at{j}", name=f"at{j}") for j in range(8)]
                    pending = [None]

                    def emit_norm(p, at_t=at_t):
                        ues_, uos_, hp_ = p
                        bce = ps.tile([128, 512], FP32, tag="pp", bufs=2)
                        bco = ps.tile([128, 512], FP32, tag="pp", bufs=2)
                        nc.tensor.matmul(
                            bce[0:64, :], ones_f[64:65, 0:64], ues_[64:65, :],
                            start=True, stop=True, tile_position=(64, 0),
                            skip_group_check=True,
                        )
                        nc.tensor.matmul(
                            bco[0:64, :], ones_f[64:65, 0:64], uos_[64:65, :],
                            start=True, stop=True, tile_position=(64, 0),
                            skip_group_check=True,
                        )
                        br1 = qp.tile([128, 512], FP32, tag="rd", bufs=4)
                        br2 = qp.tile([128, 512], FP32, tag="rd", bufs=4)
                        nc.vector.reciprocal_approx_fast(br1[0:64, :], bce[0:64, :])
                        nc.vector.reciprocal_approx_fast(br2[0:64, :], bco[0:64, :])
                        nc.vector.tensor_tensor(
                            at_t[hp_][0:64, :], ues_[0:64, :], br1[0:64, :],
                            mybir.AluOpType.mult,
                        )
                        nc.vector.tensor_tensor(
                            at_t[hp_][64:128, :], uos_[0:64, :], br2[0:64, :],
                            mybir.AluOpType.mult,
                        )

                    for hp in range(8):
                        ue = ps.tile([128, 512], FP32, tag="ue")
                        uo = ps.tile([128, 512], FP32, tag="uo")
                        Es = {}

                        def emit_sc(kt, hp=hp):
                            sc = ps.tile([128, 1024], FP32, tag="sc", bufs=2)
                            nc.tensor.matmul(
                                sc[:, 0:512], ksT[hp][0:64, kt * 128:(kt + 1) * 128],
                                qs_t[hp][0:64, :], start=True, stop=True,
                                skip_group_check=True,
                            )
                            nc.tensor.matmul(
                                sc[:, 512:1024], ksT[hp][64:128, kt * 128:(kt + 1) * 128],
                                qs_t[hp][64:128, :], start=True, stop=True,
                                tile_position=(64, 0), skip_group_check=True,
                            )
                            E = qp.tile([128, 1024], BF16, tag="E", bufs=3)
                            nc.scalar.activation(E, sc, func=EXP, bias=0.0, scale=0.125)
                            Es[kt] = E

                        emit_sc(0)
                        for kt in range(16):
                            if kt + 1 < 16:
                                emit_sc(kt + 1)
                            if kt == 6 and pending[0] is not None:
                                emit_norm(pending[0])
                                pending[0] = None
                            Ep = Es.pop(kt)
                            nc.tensor.matmul(
                                ue[0:65, :], vsp[kt][:, 130 * hp:130 * hp + 65],
                                Ep[:, 0:512],
                                start=(kt == 0), stop=(kt == 15), skip_group_check=True,
                            )
                            nc.tensor.matmul(
                                uo[0:65, :], vsp[kt][:, 130 * hp + 65:130 * hp + 130],
                                Ep[:, 512:1024],
                                start=(kt == 0), stop=(kt == 15), skip_group_check=True,
                            )
                        ues = qp.tile([128, 512], FP32, tag="ues", bufs=2)
                        uos = qp.tile([128, 512], FP32, tag="uos", bufs=2)
                        nc.vector.tensor_copy(ues[0:65, :], ue[0:65, :])
                        nc.vector.tensor_copy(uos[0:65, :], uo[0:65, :])
                        pending[0] = (ues, uos, hp)
                    emit_norm(pending[0])
                    pending[0] = None

                    lns = []
                    for rt in range(4):
                        row0 = qc * 512 + rt * 128
                        outf = qp.tile([128, D], FP32, tag="outf", bufs=4)
                        nc.sync.dma_start(outf, qn[row0:row0 + 128, :])
                        for oc in range(2):
                            po = ps.tile([128, 512], FP32, tag="pp", bufs=2)
                            for it in range(8):
                                nc.tensor.matmul(
                                    po, at_t[it][:, rt * 128:(rt + 1) * 128],
                                    wo_t[:, it, oc * 512:(oc + 1) * 512],
                                    start=(it == 0), stop=(it == 7),
                                )
                            nc.vector.tensor_add(
                                out=outf[:, oc * 512:(oc + 1) * 512],
                                in0=outf[:, oc * 512:(oc + 1) * 512], in1=po,
                            )
                        bst = qp.tile([128, 2, 6], FP32, tag="bst", bufs=2)
                        mv = qp.tile([128, 2], FP32, tag="mv", bufs=4)
                        for sg in range(2):
                            nc.vector.bn_stats(out=bst[:, sg, :], in_=outf[:, sg * 512:(sg + 1) * 512])
                        nc.vector.bn_aggr(out=mv, in_=bst)
                        lns.append((outf, mv, row0))
                    for _, mv, _r in lns:
                        nc.scalar.activation(
                            out=mv[:, 1:2], in_=mv[:, 1:2], func=SQRT,
                            bias=eps_t[:, :], scale=1.0,
                        )
                    for outf, mv, row0 in lns:
                        nc.vector.reciprocal(mv[:, 1:2], mv[:, 1:2])
                        y = qp.tile([128, D], FP32, tag="y", bufs=2)
                        nc.vector.tensor_scalar(
                            out=y, in0=outf, scalar1=mv[:, 0:1], scalar2=mv[:, 1:2],
                            op0=mybir.AluOpType.subtract, op1=mybir.AluOpType.mult,
                        )
                        nc.vector.tensor_mul(y, y, gamma_t)
                        nc.vector.tensor_add(out=y, in0=y, in1=beta_t)
                        nc.sync.dma_start(out[row0:row0 + 128, :], y)
    nc.finalize()
    return nc


def kernel(q, k, v, Wq, Wk, Wv, Wo, gamma, beta, _trace=False):
    global _NC, LAST_EXEC_NS
    if _NC is None:
        _NC = _build()
    wqh = Wq.T.astype(bf16)
    wkh = Wk.T.astype(bf16)
    wvh = Wv.T.astype(bf16)
    woh = Wo.T.astype(bf16)
    g = np.ascontiguousarray(np.asarray(gamma, dtype=np.float32).reshape(1, D))
    bt = np.ascontiguousarray(np.asarray(beta, dtype=np.float32).reshape(1, D))
    in_maps = []
    for c in range(8):
        b, hh = divmod(c, 2)
        qb = q[b, hh * NQ:(hh + 1) * NQ, :]
        in_maps.append({
            "qT": qb.T.astype(bf16),
            "qn": np.ascontiguousarray(qb, dtype=np.float32),
            "kT": k[b].T.astype(bf16),
            "vT": v[b].T.astype(bf16),
            "wq": wqh, "wk": wkh, "wv": wvh, "wo": woh,
            "gamma": g, "beta": bt,
        })
    res = bass_utils.run_bass_kernel_spmd(_NC, in_maps, list(range(8)), trace=_trace)
    LAST_EXEC_NS = getattr(res, "exec_time_ns", None)
    global LAST_RESULT
    LAST_RESULT = res
    outp = np.empty((B, N, D), np.float32)
    for c in range(8):
        b, hh = divmod(c, 2)
        outp[b, hh * NQ:(hh + 1) * NQ, :] = res.results[c]["out"]
    return outp



# revision 12
# speedup vs baseline: 1.2053x; 1.2053x over previous
import numpy as np
import ml_dtypes

import concourse.bacc as bacc
import concourse.bass as bass
import concourse.mybir as mybir
import concourse.tile as tile
from concourse import bass_utils

bf16 = ml_dtypes.bfloat16
f8 = ml_dtypes.float8_e4m3

B, N, D = 4, 2048, 1024
NQ, NK = 1024, 2048
FP32 = mybir.dt.float32
BF16 = mybir.dt.bfloat16
F8 = mybir.dt.float8e4
F8E5 = mybir.dt.float8e5
EXP = mybir.ActivationFunctionType.Exp
SQRT = mybir.ActivationFunctionType.Sqrt
DR = mybir.MatmulPerfMode.DoubleRow

LAST_EXEC_NS = None
LAST_RESULT = None
_NC = None


def _broadcast_ap(dram_ap, parts):
    return bass.AP(
        tensor=dram_ap.tensor,
        offset=dram_ap.offset,
        ap=[[0, parts], dram_ap.ap[-1]],
    )


def _build():
    nc = bacc.Bacc(None, target_bir_lowering=False)
    qT = nc.dram_tensor("qT", [D, NQ], F8, kind="ExternalInput")
    qn = nc.dram_tensor("qn", [NQ, D], FP32, kind="ExternalInput")
    kT = nc.dram_tensor("kT", [D, NK], F8, kind="ExternalInput")
    vT = nc.dram_tensor("vT", [D, NK], F8, kind="ExternalInput")
    wq = nc.dram_tensor("wq", [D, D], F8, kind="ExternalInput")
    wk = nc.dram_tensor("wk", [D, D], F8, kind="ExternalInput")
    wv = nc.dram_tensor("wv", [D, D], F8, kind="ExternalInput")
    wo = nc.dram_tensor("wo", [D, D], BF16, kind="ExternalInput")
    gamma = nc.dram_tensor("gamma", [1, D], FP32, kind="ExternalInput")
    beta = nc.dram_tensor("beta", [1, D], FP32, kind="ExternalInput")
    out = nc.dram_tensor("out", [NQ, D], FP32, kind="ExternalOutput")

    with tile.TileContext(nc) as tc:
        with (
            tc.tile_pool(name="perm", bufs=1) as perm,
            tc.tile_pool(name="qp", bufs=1) as qp,
            tc.tile_pool(name="ps", bufs=1, space="PSUM") as ps,
        ):
            gamma_t = perm.tile([128, D], FP32)
            beta_t = perm.tile([128, D], FP32)
            nc.gpsimd.dma_start(out=gamma_t, in_=_broadcast_ap(gamma[0:1, :], 128))
            nc.gpsimd.dma_start(out=beta_t, in_=_broadcast_ap(beta[0:1, :], 128))
            eps_t = perm.tile([128, 1], FP32)
            nc.vector.memset(eps_t, 1e-5)
            ones_f = perm.tile([128, 64], FP32)
            nc.vector.memset(ones_f, 1.0)
            neg2_t = perm.tile([128, 1], FP32)
            nc.vector.memset(neg2_t, -2.0)

            # weights (pre-transposed host side; (dt p) f -> p dt f)
            wk_t = perm.tile([128, 8, D], F8, name="wk_t")
            wv_t = perm.tile([128, 8, D], F8, name="wv_t")
            wq_t = perm.tile([128, 8, D], F8, name="wq_t")
            wo_t = perm.tile([128, 8, D], BF16, name="wo_t")
            nc.sync.dma_start(wk_t, wk[:, :].rearrange("(dt p) f -> p dt f", p=128))
            nc.sync.dma_start(wv_t, wv[:, :].rearrange("(dt p) f -> p dt f", p=128))
            nc.sync.dma_start(wq_t, wq[:, :].rearrange("(dt p) f -> p dt f", p=128))
            nc.sync.dma_start(wo_t, wo[:, :].rearrange("(dt p) f -> p dt f", p=128))

            # persistent activation tensors
            # ksT[hp][d, key]: d 0:64 head even, 64:128 head odd
            ksT = [perm.tile([128, NK], F8, name=f"ks{j}") for j in range(8)]
            # vsp[ktp][p, i, h, 65]: value cols 0:64 + ones col 64, key = ktp*256+i*128+p
            vsp = [perm.tile([128, 2, 16, 65], F8, name=f"vsp{t}") for t in range(8)]
            qs_t = [
                [perm.tile([128, 512], F8, name=f"qs{c}_{j}") for j in range(8)]
                for c in range(2)
            ]
            at_t = [
                [perm.tile([128, 512], BF16, name=f"at{c}_{j}") for j in range(8)]
                for c in range(2)
            ]
            outf_t = [perm.tile([128, D], FP32, name=f"outf{r}") for r in range(8)]
            mv_t = [perm.tile([128, 2], FP32, name=f"mv{r}") for r in range(8)]
            qstages = [perm.tile([128, 8, 512], F8, name=f"qstage{c}") for c in range(2)]

            def qstage_dma(qc):
                nc.sync.dma_start(
                    qstages[qc],
                    qT[:, qc * 512:(qc + 1) * 512].rearrange("(dt p) n -> p dt n", p=128),
                )

            def qproj_unit(qc, jt):
                pp = ps.tile([128, 512], FP32, tag="pp", bufs=2)
                for t in range(4):
                    nc.tensor.matmul(
                        pp,
                        wq_t[:, 2 * t:2 * t + 2, jt * 128:(jt + 1) * 128],
                        qstages[qc][:, 2 * t:2 * t + 2, :],
                        start=(t == 0), stop=(t == 3),
                        perf_mode=DR, skip_group_check=True,
                    )
                nc.vector.tensor_copy(qs_t[qc][jt], pp)

            def vproj_unit(kv_pool, rc, jc):
                vstage = kv_pool.tile([128, 8, 256], F8, tag=f"vstage{jc}", bufs=2)
                nc.gpsimd.dma_start(
                    vstage,
                    vT[:, rc * 256:(rc + 1) * 256].rearrange("(dt p) n -> p dt n", p=128),
                )
                for rt in range(2):
                    pp = ps.tile([128, 512], FP32, tag="pp", bufs=2)
                    for t in range(4):
                        nc.tensor.matmul(
                            pp,
                            vstage[:, 2 * t:2 * t + 2, rt * 128:(rt + 1) * 128],
                            wv_t[:, 2 * t:2 * t + 2, jc * 512:(jc + 1) * 512],
                            start=(t == 0), stop=(t == 3),
                            perf_mode=DR, skip_group_check=True,
                        )
                    nc.vector.tensor_copy(
                        vsp[rc][:, rt:rt + 1, jc * 8:(jc + 1) * 8, 0:64],
                        pp.rearrange("p (h c) -> p h c", h=8),
                    )
                    nc.vector.memset(vsp[rc][:, rt:rt + 1, jc * 8:(jc + 1) * 8, 64:65], 1.0)

            # ---------------- preamble: K-proj (all), V-proj jc=0, Q-proj qc=0
            with tc.tile_pool(name="kv", bufs=1) as kv:
                with nc.named_scope("kproj"):
                    for kc in range(4):
                        kstage = kv.tile([128, 8, 512], F8, tag="kstage", bufs=2)
                        nc.sync.dma_start(
                            kstage,
                            kT[:, kc * 512:(kc + 1) * 512].rearrange(
                                "(dt p) n -> p dt n", p=128
                            ),
                        )
                        for jt in range(8):
                            pp = ps.tile([128, 512], FP32, tag="pp", bufs=2)
                            for t in range(4):
                                nc.tensor.matmul(
                                    pp,
                                    wk_t[:, 2 * t:2 * t + 2, jt * 128:(jt + 1) * 128],
                                    kstage[:, 2 * t:2 * t + 2, :],
                                    start=(t == 0), stop=(t == 3),
                                    perf_mode=DR, skip_group_check=True,
                                )
                            nc.vector.tensor_copy(ksT[jt][:, kc * 512:(kc + 1) * 512], pp)
                    with nc.named_scope("vproj0"):
                        for rc in range(8):
                            vproj_unit(kv, rc, 0)
                    with nc.named_scope("qproj0"):
                        qstage_dma(0)
                        for jt in range(8):
                            qproj_unit(0, jt)

                    # ---------------- attention + interleaved fillers
                    fillers_qc = {
                        0: (
                            [lambda rc=rc: vproj_unit(kv, rc, 1) for rc in range(8)]
                            + [lambda: qstage_dma(1)]
                            + [lambda jt=jt: qproj_unit(1, jt) for jt in range(8)]
                        ),
                        1: [],  # filled after qc0 completes (O-proj qc0)
                    }

                    def emit_fillers(lst, n):
                        for _ in range(n):
                            if lst:
                                lst.pop(0)()

                    def emit_norm(p):
                        ues_, uos_, hp_, qc = p
                        bce = ps.tile([128, 512], FP32, tag="pp", bufs=2)
                        bco = ps.tile([128, 512], FP32, tag="pp", bufs=2)
                        nc.tensor.matmul(
                            bce[0:64, :], ones_f[64:65, 0:64], ues_[64:65, :],
                            start=True, stop=True, tile_position=(64, 0),
                            skip_group_check=True,
                        )
                        nc.tensor.matmul(
                            bco[0:64, :], ones_f[64:65, 0:64], uos_[64:65, :],
                            start=True, stop=True, tile_position=(64, 0),
                            skip_group_check=True,
                        )
                        br1 = qp.tile([128, 512], FP32, tag="rd", bufs=4)
                        br2 = qp.tile([128, 512], FP32, tag="rd", bufs=4)
                        nc.vector.reciprocal_approx_fast(br1[0:64, :], bce[0:64, :])
                        nc.vector.reciprocal_approx_fast(br2[0:64, :], bco[0:64, :])
                        nc.vector.tensor_tensor(
                            at_t[qc][hp_][0:64, :], ues_[0:64, :], br1[0:64, :],
                            mybir.AluOpType.mult,
                        )
                        nc.vector.tensor_tensor(
                            at_t[qc][hp_][64:128, :], uos_[0:64, :], br2[0:64, :],
                            mybir.AluOpType.mult,
                        )

                    def oproj_unit(qc, rt):
                        idx = qc * 4 + rt
                        row0 = qc * 512 + rt * 128
                        outf = outf_t[idx]
                        nc.sync.dma_start(outf, qn[row0:row0 + 128, :])
                        for oc in range(2):
                            po = ps.tile([128, 512], FP32, tag="pp", bufs=2)
                            for it in range(8):
                                nc.tensor.matmul(
                                    po, at_t[qc][it][:, rt * 128:(rt + 1) * 128],
                                    wo_t[:, it, oc * 512:(oc + 1) * 512],
                                    start=(it == 0), stop=(it == 7),
                                    skip_group_check=True,
                                )
                            nc.vector.tensor_add(
                                out=outf[:, oc * 512:(oc + 1) * 512],
                                in0=outf[:, oc * 512:(oc + 1) * 512], in1=po,
                            )
                        bst = qp.tile([128, 2, 6], FP32, tag="bst", bufs=2)
                        for sg in range(2):
                            nc.vector.bn_stats(
                                out=bst[:, sg, :], in_=outf[:, sg * 512:(sg + 1) * 512]
                            )
                        nc.vector.bn_aggr(out=mv_t[idx], in_=bst)

                    def ln_finish(idx):
                        row0 = (idx // 4) * 512 + (idx % 4) * 128
                        mv = mv_t[idx]
                        outf = outf_t[idx]
                        nc.vector.reciprocal(mv[:, 1:2], mv[:, 1:2])
                        y = qp.tile([128, D], FP32, tag="y", bufs=4)
                        nc.vector.tensor_scalar(
                            out=y, in0=outf, scalar1=mv[:, 0:1], scalar2=mv[:, 1:2],
                            op0=mybir.AluOpType.subtract, op1=mybir.AluOpType.mult,
                        )
                        nc.vector.tensor_mul(y, y, gamma_t)
                        nc.vector.tensor_add(out=y, in0=y, in1=beta_t)
                        eng = nc.sync if idx % 2 == 0 else nc.gpsimd
                        eng.dma_start(out[row0:row0 + 128, :], y)

                    pending = [None]
                    for qc in range(2):
                        with nc.named_scope(f"attn{qc}"):
                            for hp in range(8):
                                emit_fillers(fillers_qc[qc], 2)
                                ue = ps.tile([128, 512], FP32, tag="ue")
                                uo = ps.tile([128, 512], FP32, tag="uo")
                                Es = {}

                                def emit_sc(kt, hp=hp, qc=qc, Es=Es):
                                    sc = ps.tile([128, 1024], FP32, tag="sc", bufs=2)
                                    nc.tensor.matmul(
                                        sc[:, 0:512],
                                        ksT[hp][0:64, kt * 128:(kt + 1) * 128],
                                        qs_t[qc][hp][0:64, :], start=True, stop=True,
                                        skip_group_check=True,
                                    )
                                    nc.tensor.matmul(
                                        sc[:, 512:1024],
                                        ksT[hp][64:128, kt * 128:(kt + 1) * 128],
                                        qs_t[qc][hp][64:128, :], start=True, stop=True,
                                        tile_position=(64, 0), skip_group_check=True,
                                    )
                                    if kt % 2 == 0:
                                        E = qp.tile([128, 2, 2, 512], F8E5, tag="E", bufs=3)
                                        Es[kt // 2] = E
                                    else:
                                        E = Es[kt // 2]
                                    i = kt % 2
                                    nc.scalar.activation(
                                        E[:, i:i + 1, :, :], sc, func=EXP,
                                        bias=neg2_t[:, :], scale=0.125,
                                    )

                                emit_sc(0)
                                emit_sc(1)
                                for ktp in range(8):
                                    if 2 * ktp + 2 < 16:
                                        emit_sc(2 * ktp + 2)
                                    if 2 * ktp + 3 < 16:
                                        emit_sc(2 * ktp + 3)
                                    if ktp == 3 and pending[0] is not None:
                                        emit_norm(pending[0])
                                        pending[0] = None
                                    if ktp == 5:
                                        emit_fillers(fillers_qc[qc], 1)
                                    Ep = Es.pop(ktp)
                                    nc.tensor.matmul(
                                        ue[0:65, :],
                                        vsp[ktp][:, :, 2 * hp:2 * hp + 1, :],
                                        Ep[:, :, 0:1, :],
                                        start=(ktp == 0), stop=(ktp == 7),
                                        perf_mode=DR, skip_group_check=True,
                                    )
                                    nc.tensor.matmul(
                                        uo[0:65, :],
                                        vsp[ktp][:, :, 2 * hp + 1:2 * hp + 2, :],
                                        Ep[:, :, 1:2, :],
                                        start=(ktp == 0), stop=(ktp == 7),
                                        perf_mode=DR, skip_group_check=True,
                                    )
                                ues = qp.tile([128, 512], FP32, tag="ues", bufs=2)
                                uos = qp.tile([128, 512], FP32, tag="uos", bufs=2)
                                nc.vector.tensor_copy(ues[0:65, :], ue[0:65, :])
                                nc.vector.tensor_copy(uos[0:65, :], uo[0:65, :])
                                pending[0] = (ues, uos, hp, qc)
                            # end of qc: finish last norm so O-proj can run
                            emit_norm(pending[0])
                            pending[0] = None
                        if qc == 0:
                            fillers_qc[1] = [
                                lambda rt=rt: oproj_unit(0, rt) for rt in range(4)
                            ]

                    with nc.named_scope("tail"):
                        for rt in range(4):
                            oproj_unit(1, rt)
                        for idx in range(8):
                            nc.scalar.activation(
                                out=mv_t[idx][:, 1:2], in_=mv_t[idx][:, 1:2], func=SQRT,
                                bias=eps_t[:, :], scale=1.0,
                            )
                        for idx in range(8):
                            ln_finish(idx)
    nc.finalize()
    return nc


def kernel(q, k, v, Wq, Wk, Wv, Wo, gamma, beta, _trace=False):
    global _NC, LAST_EXEC_NS, LAST_RESULT
    if _NC is None:
        _NC = _build()
    wqh = Wq.T.astype(f8)
    wkh = Wk.T.astype(f8)
    wvh = Wv.T.astype(f8)
    woh = Wo.T.astype(bf16)
    g = np.ascontiguousarray(np.asarray(gamma, dtype=np.float32).reshape(1, D))
    bt = np.ascontiguousarray(np.asarray(beta, dtype=np.float32).reshape(1, D))
    in_maps = []
    for c in range(8):
        b, hh = divmod(c, 2)
        qb = q[b, hh * NQ:(hh + 1) * NQ, :]
        in_maps.append({
            "qT": qb.T.astype(f8),
            "qn": np.ascontiguousarray(qb, dtype=np.float32),
            "kT": k[b].T.astype(f8),
            "vT": v[b].T.astype(f8),
            "wq": wqh, "wk": wkh, "wv": wvh, "wo": woh,
            "gamma": g, "beta": bt,
        })
    res = bass_utils.run_bass_kernel_spmd(_NC, in_maps, list(range(8)), trace=_trace)
    LAST_EXEC_NS = getattr(res, "exec_time_ns", None)
    LAST_RESULT = res
    outp = np.empty((B, N, D), np.float32)
    for c in range(8):
        b, hh = divmod(c, 2)
        outp[b, hh * NQ:(hh + 1) * NQ, :] = res.results[c]["out"]
    return outp


# revision 14
# speedup vs baseline: 1.2229x; 1.0146x over previous
import numpy as np
import ml_dtypes

import concourse.bacc as bacc
import concourse.bass as bass
import concourse.mybir as mybir
import concourse.tile as tile
from concourse import bass_utils

bf16 = ml_dtypes.bfloat16
f8 = ml_dtypes.float8_e4m3

B, N, D = 4, 2048, 1024
NQ, NK = 1024, 2048
FP32 = mybir.dt.float32
BF16 = mybir.dt.bfloat16
F8 = mybir.dt.float8e4
F8E5 = mybir.dt.float8e5
EXP = mybir.ActivationFunctionType.Exp
SQRT = mybir.ActivationFunctionType.Sqrt
DR = mybir.MatmulPerfMode.DoubleRow

LAST_EXEC_NS = None
LAST_RESULT = None
_NC = None


def _broadcast_ap(dram_ap, parts):
    return bass.AP(
        tensor=dram_ap.tensor,
        offset=dram_ap.offset,
        ap=[[0, parts], dram_ap.ap[-1]],
    )


def _build():
    nc = bacc.Bacc(None, target_bir_lowering=False)
    qT = nc.dram_tensor("qT", [D, NQ], F8, kind="ExternalInput")
    qn = nc.dram_tensor("qn", [NQ, D], FP32, kind="ExternalInput")
    kT = nc.dram_tensor("kT", [D, NK], F8, kind="ExternalInput")
    vT = nc.dram_tensor("vT", [D, NK], F8, kind="ExternalInput")
    wq = nc.dram_tensor("wq", [D, D], F8, kind="ExternalInput")
    wk = nc.dram_tensor("wk", [D, D], F8, kind="ExternalInput")
    wv = nc.dram_tensor("wv", [D, D], F8, kind="ExternalInput")
    wo = nc.dram_tensor("wo", [D, D], BF16, kind="ExternalInput")
    gamma = nc.dram_tensor("gamma", [1, D], FP32, kind="ExternalInput")
    beta = nc.dram_tensor("beta", [1, D], FP32, kind="ExternalInput")
    out = nc.dram_tensor("out", [NQ, D], FP32, kind="ExternalOutput")

    with tile.TileContext(nc) as tc:
        with (
            tc.tile_pool(name="perm", bufs=1) as perm,
            tc.tile_pool(name="qp", bufs=1) as qp,
            tc.tile_pool(name="ps", bufs=1, space="PSUM") as ps,
        ):
            gamma_t = perm.tile([128, D], FP32)
            beta_t = perm.tile([128, D], FP32)
            nc.gpsimd.dma_start(out=gamma_t, in_=_broadcast_ap(gamma[0:1, :], 128))
            nc.gpsimd.dma_start(out=beta_t, in_=_broadcast_ap(beta[0:1, :], 128))
            eps_t = perm.tile([128, 1], FP32)
            nc.vector.memset(eps_t, 1e-5)
            ones_f = perm.tile([128, 64], FP32)
            nc.vector.memset(ones_f, 1.0)
            neg2_t = perm.tile([128, 1], FP32)
            nc.vector.memset(neg2_t, -2.0)

            # weights (pre-transposed host side; (dt p) f -> p dt f)
            wk_t = perm.tile([128, 8, D], F8, name="wk_t")
            wv_t = perm.tile([128, 8, D], F8, name="wv_t")
            wq_t = perm.tile([128, 8, D], F8, name="wq_t")
            wo_t = perm.tile([128, 8, D], BF16, name="wo_t")
            nc.sync.dma_start(wk_t, wk[:, :].rearrange("(dt p) f -> p dt f", p=128))
            nc.gpsimd.dma_start(wv_t, wv[:, :].rearrange("(dt p) f -> p dt f", p=128))
            nc.gpsimd.dma_start(wq_t, wq[:, :].rearrange("(dt p) f -> p dt f", p=128))

            # persistent activation tensors
            # ksT[hp][d, key]: d 0:64 head even, 64:128 head odd
            ksT = [perm.tile([128, NK], BF16, name=f"ks{j}") for j in range(8)]
            # vsp[ktp][p, i, h, 65]: value cols 0:64 + ones col 64, key = ktp*256+i*128+p
            vsp = [perm.tile([128, 2, 16, 65], F8, name=f"vsp{t}") for t in range(8)]
            qs_t = [
                [perm.tile([128, 512], BF16, name=f"qs{c}_{j}") for j in range(8)]
                for c in range(2)
            ]
            at_t = [
                [perm.tile([128, 512], BF16, name=f"at{c}_{j}") for j in range(8)]
                for c in range(2)
            ]
            outf_t = [perm.tile([128, D], FP32, name=f"outf{r}") for r in range(8)]
            mv_t = [perm.tile([128, 2], FP32, name=f"mv{r}") for r in range(8)]
            qstage = perm.tile([128, 8, 512], F8, name="qstage")

            def qstage_dma(qc):
                nc.sync.dma_start(
                    qstage,
                    qT[:, qc * 512:(qc + 1) * 512].rearrange("(dt p) n -> p dt n", p=128),
                )

            def qproj_unit(qc, jt):
                pp = ps.tile([128, 512], FP32, tag="pp", bufs=2)
                for t in range(4):
                    nc.tensor.matmul(
                        pp,
                        wq_t[:, 2 * t:2 * t + 2, jt * 128:(jt + 1) * 128],
                        qstage[:, 2 * t:2 * t + 2, :],
                        start=(t == 0), stop=(t == 3),
                        perf_mode=DR, skip_group_check=True,
                    )
                nc.vector.tensor_copy(qs_t[qc][jt], pp)

            def vproj_unit(kv_pool, rc, jc):
                vstage = kv_pool.tile([128, 8, 256], F8, tag=f"vstage{jc}", bufs=2)
                nc.gpsimd.dma_start(
                    vstage,
                    vT[:, rc * 256:(rc + 1) * 256].rearrange("(dt p) n -> p dt n", p=128),
                )
                for rt in range(2):
                    pp = ps.tile([128, 512], FP32, tag="pp", bufs=2)
                    for t in range(4):
                        nc.tensor.matmul(
                            pp,
                            vstage[:, 2 * t:2 * t + 2, rt * 128:(rt + 1) * 128],
                            wv_t[:, 2 * t:2 * t + 2, jc * 512:(jc + 1) * 512],
                            start=(t == 0), stop=(t == 3),
                            perf_mode=DR, skip_group_check=True,
                        )
                    nc.vector.tensor_copy(
                        vsp[rc][:, rt:rt + 1, jc * 8:(jc + 1) * 8, 0:64],
                        pp.rearrange("p (h c) -> p h c", h=8),
                    )
                    nc.vector.memset(vsp[rc][:, rt:rt + 1, jc * 8:(jc + 1) * 8, 64:65], 1.0)

            # ---------------- preamble: K-proj (all), V-proj jc=0, Q-proj qc=0
            with tc.tile_pool(name="kv", bufs=1) as kv:
                with nc.named_scope("kproj"):
                    for kc in range(4):
                        kstage = kv.tile([128, 8, 512], F8, tag="kstage", bufs=2)
                        nc.sync.dma_start(
                            kstage,
                            kT[:, kc * 512:(kc + 1) * 512].rearrange(
                                "(dt p) n -> p dt n", p=128
                            ),
                        )
                        for jt in range(8):
                            pp = ps.tile([128, 512], FP32, tag="pp", bufs=2)
                            for t in range(4):
                                nc.tensor.matmul(
                                    pp,
                                    wk_t[:, 2 * t:2 * t + 2, jt * 128:(jt + 1) * 128],
                                    kstage[:, 2 * t:2 * t + 2, :],
                                    start=(t == 0), stop=(t == 3),
                                    perf_mode=DR, skip_group_check=True,
                                )
                            nc.vector.tensor_copy(ksT[jt][:, kc * 512:(kc + 1) * 512], pp)
                    with nc.named_scope("vproj0"):
                        for rc in range(8):
                            vproj_unit(kv, rc, 0)
                    with nc.named_scope("qproj0"):
                        qstage_dma(0)
                        for jt in range(8):
                            qproj_unit(0, jt)

                    # ---------------- attention + interleaved fillers
                    def wo_dma():
                        nc.sync.dma_start(
                            wo_t, wo[:, :].rearrange("(dt p) f -> p dt f", p=128)
                        )

                    fillers_qc = {
                        0: (
                            [lambda rc=rc: vproj_unit(kv, rc, 1) for rc in range(8)]
                            + [lambda: qstage_dma(1), wo_dma]
                            + [lambda jt=jt: qproj_unit(1, jt) for jt in range(8)]
                        ),
                        1: [],  # filled after qc0 completes (O-proj qc0)
                    }

                    def emit_fillers(lst, n):
                        for _ in range(n):
                            if lst:
                                lst.pop(0)()

                    def emit_norm(p):
                        ues_, uos_, hp_, qc = p
                        bce = ps.tile([128, 512], FP32, tag="pp", bufs=2)
                        bco = ps.tile([128, 512], FP32, tag="pp", bufs=2)
                        nc.tensor.matmul(
                            bce[0:64, :], ones_f[64:65, 0:64], ues_[64:65, :],
                            start=True, stop=True, tile_position=(64, 0),
                            skip_group_check=True,
                        )
                        nc.tensor.matmul(
                            bco[0:64, :], ones_f[64:65, 0:64], uos_[64:65, :],
                            start=True, stop=True, tile_position=(64, 0),
                            skip_group_check=True,
                        )
                        br1 = qp.tile([128, 512], FP32, tag="rd", bufs=2)
                        br2 = qp.tile([128, 512], FP32, tag="rd", bufs=2)
                        nc.vector.reciprocal_approx_fast(br1[0:64, :], bce[0:64, :])
                        nc.vector.reciprocal_approx_fast(br2[0:64, :], bco[0:64, :])
                        nc.vector.tensor_tensor(
                            at_t[qc][hp_][0:64, :], ues_[0:64, :], br1[0:64, :],
                            mybir.AluOpType.mult,
                        )
                        nc.vector.tensor_tensor(
                            at_t[qc][hp_][64:128, :], uos_[0:64, :], br2[0:64, :],
                            mybir.AluOpType.mult,
                        )

                    def oproj_unit(qc, rt):
                        idx = qc * 4 + rt
                        row0 = qc * 512 + rt * 128
                        outf = outf_t[idx]
                        nc.sync.dma_start(outf, qn[row0:row0 + 128, :])
                        for oc in range(2):
                            po = ps.tile([128, 512], FP32, tag="pp", bufs=2)
                            for it in range(8):
                                nc.tensor.matmul(
                                    po, at_t[qc][it][:, rt * 128:(rt + 1) * 128],
                                    wo_t[:, it, oc * 512:(oc + 1) * 512],
                                    start=(it == 0), stop=(it == 7),
                                    skip_group_check=True,
                                )
                            nc.vector.tensor_add(
                                out=outf[:, oc * 512:(oc + 1) * 512],
                                in0=outf[:, oc * 512:(oc + 1) * 512], in1=po,
                            )
                        bst = qp.tile([128, 2, 6], FP32, tag="bst", bufs=2)
                        for sg in range(2):
                            nc.vector.bn_stats(
                                out=bst[:, sg, :], in_=outf[:, sg * 512:(sg + 1) * 512]
                            )
                        nc.vector.bn_aggr(out=mv_t[idx], in_=bst)

                    def ln_finish(idx):
                        row0 = (idx // 4) * 512 + (idx % 4) * 128
                        mv = mv_t[idx]
                        outf = outf_t[idx]
                        nc.vector.reciprocal(mv[:, 1:2], mv[:, 1:2])
                        y = qp.tile([128, D], FP32, tag="y", bufs=2)
                        nc.vector.tensor_scalar(
                            out=y, in0=outf, scalar1=mv[:, 0:1], scalar2=mv[:, 1:2],
                            op0=mybir.AluOpType.subtract, op1=mybir.AluOpType.mult,
                        )
                        nc.vector.tensor_mul(y, y, gamma_t)
                        nc.vector.tensor_add(out=y, in0=y, in1=beta_t)
                        eng = nc.sync if idx % 2 == 0 else nc.gpsimd
                        eng.dma_start(out[row0:row0 + 128, :], y)

                    pending = [None]
                    for qc in range(2):
                        with nc.named_scope(f"attn{qc}"):
                            for hp in range(8):
                                emit_fillers(fillers_qc[qc], 2)
                                ue = ps.tile([128, 512], FP32, tag="ue")
                                uo = ps.tile([128, 512], FP32, tag="uo")
                                Es = {}

                                def emit_sc(kt, hp=hp, qc=qc, Es=Es):
                                    sc = ps.tile([128, 1024], FP32, tag="sc", bufs=2)
                                    nc.tensor.matmul(
                                        sc[:, 0:512],
                                        ksT[hp][0:64, kt * 128:(kt + 1) * 128],
                                        qs_t[qc][hp][0:64, :], start=True, stop=True,
                                        skip_group_check=True,
                                    )
                                    nc.tensor.matmul(
                                        sc[:, 512:1024],
                                        ksT[hp][64:128, kt * 128:(kt + 1) * 128],
                                        qs_t[qc][hp][64:128, :], start=True, stop=True,
                                        tile_position=(64, 0), skip_group_check=True,
                                    )
                                    if kt % 2 == 0:
                                        E = qp.tile([128, 2, 2, 512], F8E5, tag="E", bufs=2)
                                        Es[kt // 2] = E
                                    else:
                                        E = Es[kt // 2]
                                    i = kt % 2
                                    nc.scalar.activation(
                                        E[:, i:i + 1, :, :], sc, func=EXP,
                                        bias=neg2_t[:, :], scale=0.125,
                                    )

                                emit_sc(0)
                                emit_sc(1)
                                for ktp in range(8):
                                    if 2 * ktp + 2 < 16:
                                        emit_sc(2 * ktp + 2)
                                    if 2 * ktp + 3 < 16:
                                        emit_sc(2 * ktp + 3)
                                    if ktp == 3 and pending[0] is not None:
                                        emit_norm(pending[0])
                                        pending[0] = None
                                    if ktp == 5:
                                        emit_fillers(fillers_qc[qc], 1)
                                    Ep = Es.pop(ktp)
                                    nc.tensor.matmul(
                                        ue[0:65, :],
                                        vsp[ktp][:, :, 2 * hp:2 * hp + 1, :],
                                        Ep[:, :, 0:1, :],
                                        start=(ktp == 0), stop=(ktp == 7),
                                        perf_mode=DR, skip_group_check=True,
                                    )
                                    nc.tensor.matmul(
                                        uo[0:65, :],
                                        vsp[ktp][:, :, 2 * hp + 1:2 * hp + 2, :],
                                        Ep[:, :, 1:2, :],
                                        start=(ktp == 0), stop=(ktp == 7),
                                        perf_mode=DR, skip_group_check=True,
                                    )
                                ues = qp.tile([128, 512], FP32, tag="ues", bufs=2)
                                uos = qp.tile([128, 512], FP32, tag="uos", bufs=2)
                                nc.vector.tensor_copy(ues[0:65, :], ue[0:65, :])
                                nc.vector.tensor_copy(uos[0:65, :], uo[0:65, :])
                                pending[0] = (ues, uos, hp, qc)
                            # end of qc: finish last norm so O-proj can run
                            emit_norm(pending[0])
                            pending[0] = None
                        if qc == 0:
                            def sqrt_batch(lo, hi):
                                for idx in range(lo, hi):
                                    nc.scalar.activation(
                                        out=mv_t[idx][:, 1:2], in_=mv_t[idx][:, 1:2],
                                        func=SQRT, bias=eps_t[:, :], scale=1.0,
                                    )
                            fillers_qc[1] = (
                                [lambda rt=rt: oproj_unit(0, rt) for rt in range(4)]
                                + [lambda: sqrt_batch(0, 4)]
                                + [lambda idx=idx: ln_finish(idx) for idx in range(4)]
                            )

                    with nc.named_scope("tail"):
                        for rt in range(4):
                            oproj_unit(1, rt)
                        for idx in range(4, 8):
                            nc.scalar.activation(
                                out=mv_t[idx][:, 1:2], in_=mv_t[idx][:, 1:2], func=SQRT,
                                bias=eps_t[:, :], scale=1.0,
                            )
                        for idx in range(4, 8):
                            ln_finish(idx)
    nc.finalize()
    return nc


def kernel(q, k, v, Wq, Wk, Wv, Wo, gamma, beta, _trace=False):
    global _NC, LAST_EXEC_NS, LAST_RESULT
    if _NC is None:
        _NC = _build()
    wqh = Wq.T.astype(f8)
    wkh = Wk.T.astype(f8)
    wvh = Wv.T.astype(f8)
    woh = Wo.T.astype(bf16)
    g = np.ascontiguousarray(np.asarray(gamma, dtype=np.float32).reshape(1, D))
    bt = np.ascontiguousarray(np.asarray(beta, dtype=np.float32).reshape(1, D))
    in_maps = []
    for c in range(8):
        b, hh = divmod(c, 2)
        qb = q[b, hh * NQ:(hh + 1) * NQ, :]
        in_maps.append({
            "qT": qb.T.astype(f8),
            "qn": np.ascontiguousarray(qb, dtype=np.float32),
            "kT": k[b].T.astype(f8),
            "vT": v[b].T.astype(f8),
            "wq": wqh, "wk": wkh, "wv": wvh, "wo": woh,
            "gamma": g, "beta": bt,
        })
    res = bass_utils.run_bass_kernel_spmd(_NC, in_maps, list(range(8)), trace=_trace)
    LAST_EXEC_NS = getattr(res, "exec_time_ns", None)
    LAST_RESULT = res
    outp = np.empty((B, N, D), np.float32)
    for c in range(8):
        b, hh = divmod(c, 2)
        outp[b, hh * NQ:(hh + 1) * NQ, :] = res.results[c]["out"]
    return outp


# revision 21
# speedup vs baseline: 1.2823x; 1.0485x over previous
import numpy as np
import ml_dtypes

import concourse.bacc as bacc
import concourse.bass as bass
import concourse.mybir as mybir
import concourse.tile as tile
from concourse import bass_utils

bf16 = ml_dtypes.bfloat16
f8 = ml_dtypes.float8_e4m3

B, N, D = 4, 2048, 1024
NQ, NK = 1024, 2048
FP32 = mybir.dt.float32
BF16 = mybir.dt.bfloat16
F8 = mybir.dt.float8e4
F8E5 = mybir.dt.float8e5
EXP = mybir.ActivationFunctionType.Exp
SQRT = mybir.ActivationFunctionType.Sqrt
DR = mybir.MatmulPerfMode.DoubleRow

LAST_EXEC_NS = None
LAST_RESULT = None
_NC = None


def _broadcast_ap(dram_ap, parts):
    return bass.AP(
        tensor=dram_ap.tensor,
        offset=dram_ap.offset,
        ap=[[0, parts], dram_ap.ap[-1]],
    )


def _build():
    nc = bacc.Bacc(None, target_bir_lowering=False)
    # host-packed SBUF-image inputs (contiguous per partition)
    kS = nc.dram_tensor("kS", [128, 4, 8, 512], F8, kind="ExternalInput")
    vS = nc.dram_tensor("vS", [128, 8, 8, 256], F8, kind="ExternalInput")
    qS = nc.dram_tensor("qS", [128, 2, 8, 512], F8, kind="ExternalInput")
    wkp = nc.dram_tensor("wkp", [128, 8, D], F8, kind="ExternalInput")
    wvp = nc.dram_tensor("wvp", [128, 8, D], F8, kind="ExternalInput")
    wqp = nc.dram_tensor("wqp", [128, 8, D], F8, kind="ExternalInput")
    wop = nc.dram_tensor("wop", [128, 8, D], BF16, kind="ExternalInput")
    qn = nc.dram_tensor("qn", [NQ, D], FP32, kind="ExternalInput")
    gamma = nc.dram_tensor("gamma", [1, D], FP32, kind="ExternalInput")
    beta = nc.dram_tensor("beta", [1, D], FP32, kind="ExternalInput")
    out = nc.dram_tensor("out", [NQ, D], FP32, kind="ExternalOutput")

    with tile.TileContext(nc) as tc:
        with (
            tc.tile_pool(name="perm", bufs=1) as perm,
            tc.tile_pool(name="qp", bufs=1) as qp,
            tc.tile_pool(name="ps", bufs=1, space="PSUM") as ps,
        ):
            gamma_t = perm.tile([128, D], BF16)
            beta_t = perm.tile([128, D], BF16)
            nc.gpsimd.dma_start(out=gamma_t, in_=_broadcast_ap(gamma[0:1, :], 128))
            nc.gpsimd.dma_start(out=beta_t, in_=_broadcast_ap(beta[0:1, :], 128))
            eps_t = perm.tile([128, 1], FP32)
            nc.vector.memset(eps_t, 1e-5)
            ones_f = perm.tile([128, 64], FP32)
            nc.vector.memset(ones_f, 1.0)
            neg2_t = perm.tile([128, 1], FP32)
            nc.vector.memset(neg2_t, -2.0)

            wk_t = perm.tile([128, 8, D], F8, name="wk_t")
            wv_t = perm.tile([128, 8, D], F8, name="wv_t")
            wq_t = perm.tile([128, 8, D], F8, name="wq_t")
            wo_t = perm.tile([128, 8, D], BF16, name="wo_t")
            nc.sync.dma_start(wk_t, wkp[:, :, :])
            nc.gpsimd.dma_start(wv_t, wvp[:, :, :])
            nc.gpsimd.dma_start(wq_t, wqp[:, :, :])

            kstage = perm.tile([128, 4, 8, 512], F8, name="kstage")
            nc.sync.dma_start(kstage[:, 0:2, :, :], kS[:, 0:2, :, :])
            nc.sync.dma_start(kstage[:, 2:4, :, :], kS[:, 2:4, :, :])
            qstage = perm.tile([128, 8, 512], F8, name="qstage")
            nc.scalar.dma_start(qstage, qS[:, 0, :, :])

            # persistent activation tensors
            ksT = [perm.tile([128, NK], BF16, name=f"ks{j}") for j in range(8)]
            vsp = [perm.tile([128, 2, 16, 65], F8, name=f"vsp{t}") for t in range(8)]
            qs_t = [
                [perm.tile([128, 512], BF16, name=f"qs{c}_{j}") for j in range(8)]
                for c in range(2)
            ]
            at_t = [
                [perm.tile([128, 512], BF16, name=f"at{c}_{j}") for j in range(8)]
                for c in range(2)
            ]
            outf_t = [perm.tile([128, D], FP32, name=f"outf{r}") for r in range(8)]
            mv_t = [perm.tile([128, 2], FP32, name=f"mv{r}") for r in range(8)]

            def ksub(jt, kc):
                pp = ps.tile([128, 512], FP32, tag="pp", bufs=2)
                for t in range(4):
                    nc.tensor.matmul(
                        pp,
                        wk_t[:, 2 * t:2 * t + 2, jt * 128:(jt + 1) * 128],
                        kstage[:, kc, 2 * t:2 * t + 2, :],
                        start=(t == 0), stop=(t == 3),
                        perf_mode=DR, skip_group_check=True,
                    )
                nc.vector.tensor_copy(ksT[jt][:, kc * 512:(kc + 1) * 512], pp)

            def kproj_unit(jt):
                for kc in range(4):
                    ksub(jt, kc)

            def qstage_dma(qc):
                nc.sync.dma_start(qstage, qS[:, qc, :, :])

            def qproj_unit(qc, jt):
                pp = ps.tile([128, 512], FP32, tag="pp", bufs=2)
                for t in range(4):
                    nc.tensor.matmul(
                        pp,
                        wq_t[:, 2 * t:2 * t + 2, jt * 128:(jt + 1) * 128],
                        qstage[:, 2 * t:2 * t + 2, :],
                        start=(t == 0), stop=(t == 3),
                        perf_mode=DR, skip_group_check=True,
                    )
                nc.vector.tensor_copy(qs_t[qc][jt], pp)

            vstages = {}

            def vsub(rc, jc, rt):
                if rt == 0:
                    vstages[(rc, jc)] = qp.tile(
                        [128, 8, 256], F8, tag="vstage", bufs=2, name=f"vst{rc}_{jc}"
                    )
                    nc.gpsimd.dma_start(vstages[(rc, jc)], vS[:, rc, :, :])
                vstage = vstages[(rc, jc)]
                pp = ps.tile([128, 512], FP32, tag="pp", bufs=2)
                for t in range(4):
                    nc.tensor.matmul(
                        pp,
                        vstage[:, 2 * t:2 * t + 2, rt * 128:(rt + 1) * 128],
                        wv_t[:, 2 * t:2 * t + 2, jc * 512:(jc + 1) * 512],
                        start=(t == 0), stop=(t == 3),
                        perf_mode=DR, skip_group_check=True,
                    )
                nc.vector.tensor_copy(
                    vsp[rc][:, rt:rt + 1, jc * 8:(jc + 1) * 8, 0:64],
                    pp.rearrange("p (h c) -> p h c", h=8),
                )
                nc.vector.memset(vsp[rc][:, rt:rt + 1, jc * 8:(jc + 1) * 8, 64:65], 1.0)

            def vproj_unit(rc, jc):
                vsub(rc, jc, 0)
                vsub(rc, jc, 1)

            def wo_dma():
                nc.sync.dma_start(wo_t, wop[:, :, :])

            def emit_norm(p):
                ues_, uos_, hp_, qc_ = p
                bce = ps.tile([128, 512], FP32, tag="pp", bufs=2)
                bco = ps.tile([128, 512], FP32, tag="pp", bufs=2)
                nc.tensor.matmul(
                    bce[0:64, :], ones_f[64:65, 0:64], ues_[64:65, :],
                    start=True, stop=True, tile_position=(64, 0),
                    skip_group_check=True,
                )
                nc.tensor.matmul(
                    bco[0:64, :], ones_f[64:65, 0:64], uos_[64:65, :],
                    start=True, stop=True, tile_position=(64, 0),
                    skip_group_check=True,
                )
                br1 = qp.tile([128, 512], FP32, tag="rd", bufs=2)
                br2 = qp.tile([128, 512], FP32, tag="rd", bufs=2)
                nc.vector.reciprocal_approx_fast(br1[0:64, :], bce[0:64, :])
                nc.vector.reciprocal_approx_fast(br2[0:64, :], bco[0:64, :])
                nc.vector.tensor_tensor(
                    at_t[qc_][hp_][0:64, :], ues_[0:64, :], br1[0:64, :],
                    mybir.AluOpType.mult,
                )
                nc.vector.tensor_tensor(
                    at_t[qc_][hp_][64:128, :], uos_[0:64, :], br2[0:64, :],
                    mybir.AluOpType.mult,
                )

            def qn_dma(qc, rt):
                idx = qc * 4 + rt
                row0 = qc * 512 + rt * 128
                nc.sync.dma_start(outf_t[idx], qn[row0:row0 + 128, :])

            def osub(qc, rt, oc, it0, it1):
                idx = qc * 4 + rt
                outf = outf_t[idx]
                po = ps.tile([128, 512], FP32, tag="pp", bufs=2)
                for it in range(it0, it1):
                    nc.tensor.matmul(
                        po, at_t[qc][it][:, rt * 128:(rt + 1) * 128],
                        wo_t[:, it, oc * 512:(oc + 1) * 512],
                        start=(it == it0), stop=(it == it1 - 1),
                        skip_group_check=True,
                    )
                nc.vector.tensor_add(
                    out=outf[:, oc * 512:(oc + 1) * 512],
                    in0=outf[:, oc * 512:(oc + 1) * 512], in1=po,
                )

            def obn(qc, rt):
                idx = qc * 4 + rt
                outf = outf_t[idx]
                bst = qp.tile([128, 2, 6], FP32, tag="bst", bufs=2)
                for sg in range(2):
                    nc.vector.bn_stats(
                        out=bst[:, sg, :], in_=outf[:, sg * 512:(sg + 1) * 512]
                    )
                nc.vector.bn_aggr(out=mv_t[idx], in_=bst)

            def oproj_unit(qc, rt):
                qn_dma(qc, rt)
                osub(qc, rt, 0, 0, 8)
                osub(qc, rt, 1, 0, 8)
                obn(qc, rt)

            def sqrt_batch(lo, hi, bias_t):
                for idx in range(lo, hi):
                    nc.scalar.activation(
                        out=mv_t[idx][:, 1:2], in_=mv_t[idx][:, 1:2],
                        func=SQRT, bias=bias_t[:, :], scale=1.0,
                    )

            def ln_finish(idx):
                row0 = (idx // 4) * 512 + (idx % 4) * 128
                mv = mv_t[idx]
                outf = outf_t[idx]
                nc.vector.reciprocal(mv[:, 1:2], mv[:, 1:2])
                y = qp.tile([128, D], FP32, tag="y", bufs=2)
                nc.vector.tensor_scalar(
                    out=y, in0=outf, scalar1=mv[:, 0:1], scalar2=mv[:, 1:2],
                    op0=mybir.AluOpType.subtract, op1=mybir.AluOpType.mult,
                )
                nc.vector.tensor_mul(y, y, gamma_t)
                nc.vector.tensor_add(out=y, in0=y, in1=beta_t)
                eng = nc.sync if idx % 2 == 0 else nc.gpsimd
                eng.dma_start(out[row0:row0 + 128, :], y)

            # ---------------- preamble compute
            with nc.named_scope("preamble"):
                kproj_unit(0)
                vproj_unit(0, 0)
                qproj_unit(0, 0)
                kproj_unit(1)

            # ---------------- explicit filler schedule (sub-unit granularity)
            # consumed one per ktp slot in order; leftovers emitted at unit end
            K = lambda jt, kc: (lambda: ksub(jt, kc))
            Q = lambda qc, jt: (lambda: qproj_unit(qc, jt))
            V = lambda rc, jc, rt: (lambda: vsub(rc, jc, rt))
            FILL = {}
            FILL[(0, 0)] = ([lambda rc=rc: vproj_unit(rc, 0) for rc in range(1, 8)]
                            + [Q(0, 1)])
            FILL[(0, 1)] = [Q(0, 2), V(0, 1, 0), V(0, 1, 1), V(1, 1, 0), V(1, 1, 1),
                            K(2, 0), K(2, 1), K(2, 2), K(2, 3)]
            FILL[(0, 2)] = [Q(0, 3), V(2, 1, 0), V(2, 1, 1), V(3, 1, 0), V(3, 1, 1),
                            K(3, 0), K(3, 1), K(3, 2), K(3, 3)]
            FILL[(0, 3)] = [Q(0, 4), V(4, 1, 0), V(4, 1, 1), V(5, 1, 0), V(5, 1, 1),
                            K(4, 0), K(4, 1), K(4, 2), K(4, 3)]
            FILL[(0, 4)] = [Q(0, 5), V(6, 1, 0), V(6, 1, 1), V(7, 1, 0), V(7, 1, 1),
                            K(5, 0), K(5, 1), K(5, 2), K(5, 3)]
            FILL[(0, 5)] = [Q(0, 6), Q(0, 7), K(6, 0), K(6, 1), K(6, 2), K(6, 3),
                            lambda: qstage_dma(1)]
            FILL[(0, 6)] = [Q(1, 0), Q(1, 1), Q(1, 2), K(7, 0), K(7, 1), K(7, 2),
                            K(7, 3), Q(1, 3)]
            FILL[(0, 7)] = [Q(1, 4), Q(1, 5), Q(1, 6), Q(1, 7), wo_dma]
            FILL[(1, 0)] = [lambda: qn_dma(0, 0),
                            lambda: osub(0, 0, 0, 0, 4), lambda: osub(0, 0, 0, 4, 8),
                            lambda: osub(0, 0, 1, 0, 4), lambda: osub(0, 0, 1, 4, 8),
                            lambda: obn(0, 0)]
            FILL[(1, 1)] = [lambda: qn_dma(0, 1),
                            lambda: osub(0, 1, 0, 0, 4), lambda: osub(0, 1, 0, 4, 8),
                            lambda: osub(0, 1, 1, 0, 4), lambda: osub(0, 1, 1, 4, 8),
                            lambda: obn(0, 1)]
            FILL[(1, 2)] = [lambda: qn_dma(0, 2),
                            lambda: osub(0, 2, 0, 0, 4), lambda: osub(0, 2, 0, 4, 8),
                            lambda: osub(0, 2, 1, 0, 4), lambda: osub(0, 2, 1, 4, 8),
                            lambda: obn(0, 2)]
            FILL[(1, 3)] = [lambda: qn_dma(0, 3),
                            lambda: osub(0, 3, 0, 0, 4), lambda: osub(0, 3, 0, 4, 8),
                            lambda: osub(0, 3, 1, 0, 4), lambda: osub(0, 3, 1, 4, 8),
                            lambda: obn(0, 3)]

            # gate for LN(qc0) sqrt ops: ready only once all four mv are done
            eps3_t = qp.tile([128, 1], FP32, tag="eps3", bufs=1)

            def ln0_gate():
                nc.vector.tensor_scalar(
                    out=eps3_t, in0=mv_t[3][:, 0:1], scalar1=0.0, scalar2=1e-5,
                    op0=mybir.AluOpType.mult, op1=mybir.AluOpType.add,
                )

            FILL[(1, 4)] = [ln0_gate, lambda: sqrt_batch(0, 4, eps3_t),
                            lambda: ln_finish(0), lambda: ln_finish(1),
                            lambda: ln_finish(2), lambda: ln_finish(3)]
            FILL[(1, 5)] = []
            FILL[(1, 6)] = [lambda: qn_dma(1, 0), lambda: qn_dma(1, 1),
                            lambda: qn_dma(1, 2),
                            lambda: osub(1, 0, 0, 0, 6), lambda: osub(1, 0, 1, 0, 6),
                            lambda: osub(1, 1, 0, 0, 6), lambda: osub(1, 1, 1, 0, 6),
                            lambda: qn_dma(1, 3)]
            FILL[(1, 7)] = [lambda: osub(1, 2, 0, 0, 6), lambda: osub(1, 2, 1, 0, 6),
                            lambda: osub(1, 3, 0, 0, 6), lambda: osub(1, 3, 1, 0, 6)]

            # ---------------- attention
            units = [(qc, hp) for qc in range(2) for hp in range(8)]
            states = {}

            def emit_sc(u, kt):
                qc_, hp_ = u
                st = states[u]
                sc = ps.tile([128, 1024], FP32, tag="sc", bufs=2)
                nc.tensor.matmul(
                    sc[:, 0:512],
                    ksT[hp_][0:64, kt * 128:(kt + 1) * 128],
                    qs_t[qc_][hp_][0:64, :], start=True, stop=True,
                    skip_group_check=True,
                )
                nc.tensor.matmul(
                    sc[:, 512:1024],
                    ksT[hp_][64:128, kt * 128:(kt + 1) * 128],
                    qs_t[qc_][hp_][64:128, :], start=True, stop=True,
                    tile_position=(64, 0), skip_group_check=True,
                )
                if kt % 2 == 0:
                    E = qp.tile([128, 2, 2, 512], F8E5, tag="E", bufs=2)
                    st[kt // 2] = E
                else:
                    E = st[kt // 2]
                i = kt % 2
                nc.scalar.activation(
                    E[:, i:i + 1, :, :], sc, func=EXP,
                    bias=neg2_t[:, :], scale=0.125,
                )

            def prelude(u):
                states[u] = {}
                emit_sc(u, 0)
                emit_sc(u, 1)

            pending = [None]
            prelude(units[0])
            for ui, u in enumerate(units):
                qc, hp = u
                fills = list(FILL[u])
                with nc.named_scope(f"attn{qc}"):
                    ue = ps.tile([128, 512], FP32, tag="ue")
                    uo = ps.tile([128, 512], FP32, tag="uo")
                    st = states[u]
                    for ktp in range(8):
                        if 2 * ktp + 2 < 16:
                            emit_sc(u, 2 * ktp + 2)
                        if 2 * ktp + 3 < 16:
                            emit_sc(u, 2 * ktp + 3)
                        if ktp == 3 and pending[0] is not None:
                            emit_norm(pending[0])
                            pending[0] = None
                        if fills:
                            fills.pop(0)()
                        if ktp == 7 and ui + 1 < len(units):
                            prelude(units[ui + 1])
                        Ep = st.pop(ktp)
                        nc.tensor.matmul(
                            ue[0:65, :],
                            vsp[ktp][:, :, 2 * hp:2 * hp + 1, :],
                            Ep[:, :, 0:1, :],
                            start=(ktp == 0), stop=(ktp == 7),
                            perf_mode=DR, skip_group_check=True,
                        )
                        nc.tensor.matmul(
                            uo[0:65, :],
                            vsp[ktp][:, :, 2 * hp + 1:2 * hp + 2, :],
                            Ep[:, :, 1:2, :],
                            start=(ktp == 0), stop=(ktp == 7),
                            perf_mode=DR, skip_group_check=True,
                        )
                    while fills:
                        fills.pop(0)()
                    ues = qp.tile([128, 512], FP32, tag="ues", bufs=2)
                    uos = qp.tile([128, 512], FP32, tag="uos", bufs=2)
                    nc.vector.tensor_copy(ues[0:65, :], ue[0:65, :])
                    nc.vector.tensor_copy(uos[0:65, :], uo[0:65, :])
                    pending[0] = (ues, uos, hp, qc)
                    if hp == 7:
                        emit_norm(pending[0])
                        pending[0] = None

            # ---------------- tail: O-proj qc1 it6..7 + LN of qc1
            with nc.named_scope("tail"):
                for rt in range(4):
                    osub(1, rt, 0, 6, 8)
                    osub(1, rt, 1, 6, 8)
                    obn(1, rt)
                sqrt_batch(4, 8, eps_t)
                for idx in range(4, 8):
                    ln_finish(idx)
    nc.finalize()
    return nc


def _pack_w(W, dt):
    return np.ascontiguousarray(
        np.asarray(W).T.reshape(8, 128, D).transpose(1, 0, 2)).astype(dt)


def _pack_inputs(qb, kb, vb, wqh, wkh, wvh, woh, g, bt, qnb):
    kS = np.ascontiguousarray(
        np.asarray(kb).T.reshape(8, 128, 4, 512).transpose(1, 2, 0, 3)).astype(f8)
    vS = np.ascontiguousarray(
        np.asarray(vb).T.reshape(8, 128, 8, 256).transpose(1, 2, 0, 3)).astype(f8)
    qS = np.ascontiguousarray(
        np.asarray(qb).T.reshape(8, 128, 2, 512).transpose(1, 2, 0, 3)).astype(f8)
    return {
        "kS": kS, "vS": vS, "qS": qS,
        "wkp": wkh, "wvp": wvh, "wqp": wqh, "wop": woh,
        "qn": qnb, "gamma": g, "beta": bt,
    }


def kernel(q, k, v, Wq, Wk, Wv, Wo, gamma, beta, _trace=False):
    global _NC, LAST_EXEC_NS, LAST_RESULT
    if _NC is None:
        _NC = _build()
    wqh = _pack_w(Wq, f8)
    wkh = _pack_w(Wk, f8)
    wvh = _pack_w(Wv, f8)
    woh = _pack_w(Wo, bf16)
    g = np.ascontiguousarray(np.asarray(gamma, dtype=np.float32).reshape(1, D))
    bt = np.ascontiguousarray(np.asarray(beta, dtype=np.float32).reshape(1, D))
    in_maps = []
    for c in range(8):
        b, hh = divmod(c, 2)
        qb = q[b, hh * NQ:(hh + 1) * NQ, :]
        in_maps.append(_pack_inputs(
            qb, k[b], v[b], wqh, wkh, wvh, woh, g, bt,
            np.ascontiguousarray(qb, dtype=np.float32),
        ))
    res = bass_utils.run_bass_kernel_spmd(_NC, in_maps, list(range(8)), trace=_trace)
    LAST_EXEC_NS = getattr(res, "exec_time_ns", None)
    LAST_RESULT = res
    outp = np.empty((B, N, D), np.float32)
    for c in range(8):
        b, hh = divmod(c, 2)
        outp[b, hh * NQ:(hh + 1) * NQ, :] = res.results[c]["out"]
    return outp


# revision 22
# speedup vs baseline: 1.4360x; 1.1199x over previous
import numpy as np
import ml_dtypes

import concourse.bacc as bacc
import concourse.bass as bass
import concourse.mybir as mybir
import concourse.tile as tile
from concourse import bass_utils

bf16 = ml_dtypes.bfloat16
f8 = ml_dtypes.float8_e4m3

B, N, D = 4, 2048, 1024
NQ, NK = 1024, 2048
FP32 = mybir.dt.float32
BF16 = mybir.dt.bfloat16
F8 = mybir.dt.float8e4
F8E5 = mybir.dt.float8e5
EXP = mybir.ActivationFunctionType.Exp
SQRT = mybir.ActivationFunctionType.Sqrt
DR = mybir.MatmulPerfMode.DoubleRow

LAST_EXEC_NS = None
LAST_RESULT = None
_NC = None


def _broadcast_ap(dram_ap, parts):
    return bass.AP(
        tensor=dram_ap.tensor,
        offset=dram_ap.offset,
        ap=[[0, parts], dram_ap.ap[-1]],
    )


def _build(fuse_gamma=True):
    nc = bacc.Bacc(None, target_bir_lowering=False)
    # host-packed SBUF-image inputs (contiguous per partition)
    kS = nc.dram_tensor("kS", [128, 4, 8, 512], F8, kind="ExternalInput")
    vS = nc.dram_tensor("vS", [128, 8, 8, 256], F8, kind="ExternalInput")
    qS = nc.dram_tensor("qS", [128, 2, 8, 512], F8, kind="ExternalInput")
    wkp = nc.dram_tensor("wkp", [128, 8, D], F8, kind="ExternalInput")
    wvp = nc.dram_tensor("wvp", [128, 8, D], F8, kind="ExternalInput")
    wqp = nc.dram_tensor("wqp", [128, 8, D], F8, kind="ExternalInput")
    wop = nc.dram_tensor("wop", [128, 8, D], BF16, kind="ExternalInput")
    qn = nc.dram_tensor("qn", [NQ, D], FP32, kind="ExternalInput")
    gamma = nc.dram_tensor("gamma", [1, D], FP32, kind="ExternalInput")
    beta = nc.dram_tensor("beta", [1, D], FP32, kind="ExternalInput")
    out = nc.dram_tensor("out", [NQ, D], FP32, kind="ExternalOutput")

    with tile.TileContext(nc) as tc:
        with (
            tc.tile_pool(name="perm", bufs=1) as perm,
            tc.tile_pool(name="qp", bufs=1) as qp,
            tc.tile_pool(name="ps", bufs=1, space="PSUM") as ps,
        ):
            beta_t = perm.tile([128, D], FP32)
            nc.gpsimd.dma_start(out=beta_t, in_=_broadcast_ap(beta[0:1, :], 128))
            if not fuse_gamma:
                gamma_t = perm.tile([128, D], FP32)
                nc.gpsimd.dma_start(out=gamma_t, in_=_broadcast_ap(gamma[0:1, :], 128))
            eps_t = perm.tile([128, 1], FP32)
            nc.vector.memset(eps_t, 1e-5)
            ones_f = perm.tile([128, 64], BF16)
            nc.vector.memset(ones_f, 1.0)
            neg2_t = perm.tile([128, 1], FP32)
            nc.vector.memset(neg2_t, -2.0)

            wk_t = perm.tile([128, 8, D], F8, name="wk_t")
            wv_t = perm.tile([128, 8, D], F8, name="wv_t")
            wq_t = perm.tile([128, 8, D], F8, name="wq_t")
            wo_t = perm.tile([128, 8, D], BF16, name="wo_t")
            nc.sync.dma_start(wk_t, wkp[:, :, :])
            nc.gpsimd.dma_start(wv_t, wvp[:, :, :])
            nc.scalar.dma_start(wq_t, wqp[:, :, :])

            kstage = perm.tile([128, 4, 8, 512], F8, name="kstage")
            nc.sync.dma_start(kstage[:, 0:2, :, :], kS[:, 0:2, :, :])
            nc.sync.dma_start(kstage[:, 2:4, :, :], kS[:, 2:4, :, :])
            qstage = perm.tile([128, 8, 512], F8, name="qstage")
            nc.scalar.dma_start(qstage, qS[:, 0, :, :])

            # persistent activation tensors
            ksT = [perm.tile([128, NK], BF16, name=f"ks{j}") for j in range(8)]
            vsp = [perm.tile([128, 2, 16, 65], F8, name=f"vsp{t}") for t in range(8)]
            qs_t = [
                [perm.tile([128, 512], BF16, name=f"qs{c}_{j}") for j in range(8)]
                for c in range(2)
            ]
            at_t = [
                [perm.tile([128, 512], BF16, name=f"at{c}_{j}") for j in range(8)]
                for c in range(2)
            ]
            outf_t = [perm.tile([128, D], FP32, name=f"outf{r}") for r in range(8)]
            mv_t = [perm.tile([128, 2], FP32, name=f"mv{r}") for r in range(8)]

            def ksub(jt, kc):
                pp = ps.tile([128, 512], FP32, tag="pp", bufs=2)
                for t in range(4):
                    nc.tensor.matmul(
                        pp,
                        wk_t[:, 2 * t:2 * t + 2, jt * 128:(jt + 1) * 128],
                        kstage[:, kc, 2 * t:2 * t + 2, :],
                        start=(t == 0), stop=(t == 3),
                        perf_mode=DR, skip_group_check=True,
                    )
                nc.vector.tensor_copy(ksT[jt][:, kc * 512:(kc + 1) * 512], pp)

            def kproj_unit(jt):
                for kc in range(4):
                    ksub(jt, kc)

            def qstage_dma(qc):
                nc.sync.dma_start(qstage, qS[:, qc, :, :])

            def qproj_unit(qc, jt):
                pp = ps.tile([128, 512], FP32, tag="pp", bufs=2)
                for t in range(4):
                    nc.tensor.matmul(
                        pp,
                        wq_t[:, 2 * t:2 * t + 2, jt * 128:(jt + 1) * 128],
                        qstage[:, 2 * t:2 * t + 2, :],
                        start=(t == 0), stop=(t == 3),
                        perf_mode=DR, skip_group_check=True,
                    )
                nc.vector.tensor_copy(qs_t[qc][jt], pp)

            vstages = {}

            def vsub(rc, jc, rt):
                if rt == 0:
                    vstages[(rc, jc)] = qp.tile(
                        [128, 8, 256], F8, tag="vstage", bufs=2, name=f"vst{rc}_{jc}"
                    )
                    nc.gpsimd.dma_start(vstages[(rc, jc)], vS[:, rc, :, :])
                vstage = vstages[(rc, jc)]
                pp = ps.tile([128, 512], FP32, tag="pp", bufs=2)
                for t in range(4):
                    nc.tensor.matmul(
                        pp,
                        vstage[:, 2 * t:2 * t + 2, rt * 128:(rt + 1) * 128],
                        wv_t[:, 2 * t:2 * t + 2, jc * 512:(jc + 1) * 512],
                        start=(t == 0), stop=(t == 3),
                        perf_mode=DR, skip_group_check=True,
                    )
                nc.vector.tensor_copy(
                    vsp[rc][:, rt:rt + 1, jc * 8:(jc + 1) * 8, 0:64],
                    pp.rearrange("p (h c) -> p h c", h=8),
                )
                nc.vector.memset(vsp[rc][:, rt:rt + 1, jc * 8:(jc + 1) * 8, 64:65], 1.0)

            def vproj_unit(rc, jc):
                vsub(rc, jc, 0)
                vsub(rc, jc, 1)

            def wo_dma():
                nc.sync.dma_start(wo_t, wop[:, :, :])

            def emit_norm(p):
                ues_, uos_, hp_, qc_ = p
                bce = ps.tile([128, 512], FP32, tag="pp", bufs=2)
                bco = ps.tile([128, 512], FP32, tag="pp", bufs=2)
                nc.tensor.matmul(
                    bce[0:64, :], ones_f[64:65, 0:64], ues_[64:65, :],
                    start=True, stop=True, tile_position=(64, 0),
                    skip_group_check=True,
                )
                nc.tensor.matmul(
                    bco[0:64, :], ones_f[64:65, 0:64], uos_[64:65, :],
                    start=True, stop=True, tile_position=(64, 0),
                    skip_group_check=True,
                )
                br1 = qp.tile([128, 512], FP32, tag="rd", bufs=2)
                br2 = qp.tile([128, 512], FP32, tag="rd", bufs=2)
                nc.vector.reciprocal_approx_fast(br1[0:64, :], bce[0:64, :])
                nc.vector.reciprocal_approx_fast(br2[0:64, :], bco[0:64, :])
                nc.vector.tensor_tensor(
                    at_t[qc_][hp_][0:64, :], ues_[0:64, :], br1[0:64, :],
                    mybir.AluOpType.mult,
                )
                nc.vector.tensor_tensor(
                    at_t[qc_][hp_][64:128, :], uos_[0:64, :], br2[0:64, :],
                    mybir.AluOpType.mult,
                )

            def qn_dma(qc, rt):
                idx = qc * 4 + rt
                row0 = qc * 512 + rt * 128
                nc.sync.dma_start(outf_t[idx], qn[row0:row0 + 128, :])

            def osub(qc, rt, oc, it0, it1):
                idx = qc * 4 + rt
                outf = outf_t[idx]
                po = ps.tile([128, 512], FP32, tag="pp", bufs=2)
                for it in range(it0, it1):
                    nc.tensor.matmul(
                        po, at_t[qc][it][:, rt * 128:(rt + 1) * 128],
                        wo_t[:, it, oc * 512:(oc + 1) * 512],
                        start=(it == it0), stop=(it == it1 - 1),
                        skip_group_check=True,
                    )
                nc.vector.tensor_add(
                    out=outf[:, oc * 512:(oc + 1) * 512],
                    in0=outf[:, oc * 512:(oc + 1) * 512], in1=po,
                )

            def obn(qc, rt):
                idx = qc * 4 + rt
                outf = outf_t[idx]
                bst = qp.tile([128, 2, 6], FP32, tag="bst", bufs=2)
                for sg in range(2):
                    nc.vector.bn_stats(
                        out=bst[:, sg, :], in_=outf[:, sg * 512:(sg + 1) * 512]
                    )
                nc.vector.bn_aggr(out=mv_t[idx], in_=bst)

            def oproj_unit(qc, rt):
                qn_dma(qc, rt)
                osub(qc, rt, 0, 0, 8)
                osub(qc, rt, 1, 0, 8)
                obn(qc, rt)

            def sqrt_batch(lo, hi, bias_t):
                for idx in range(lo, hi):
                    nc.scalar.activation(
                        out=mv_t[idx][:, 1:2], in_=mv_t[idx][:, 1:2],
                        func=SQRT, bias=bias_t[:, :], scale=1.0,
                    )

            def ln_finish(idx):
                row0 = (idx // 4) * 512 + (idx % 4) * 128
                mv = mv_t[idx]
                outf = outf_t[idx]
                nc.vector.reciprocal(mv[:, 1:2], mv[:, 1:2])
                y = qp.tile([128, D], FP32, tag="y", bufs=2)
                if fuse_gamma:
                    nmu = qp.tile([128, 1], FP32, tag="nmu", bufs=2)
                    nc.vector.tensor_scalar(
                        out=nmu, in0=mv[:, 0:1], scalar1=mv[:, 1:2], scalar2=-1.0,
                        op0=mybir.AluOpType.mult, op1=mybir.AluOpType.mult,
                    )
                    nc.vector.affine_then_add(
                        y, outf, beta_t, scale=mv[:, 1:2], bias=nmu[:, 0:1],
                    )
                else:
                    nc.vector.tensor_scalar(
                        out=y, in0=outf, scalar1=mv[:, 0:1], scalar2=mv[:, 1:2],
                        op0=mybir.AluOpType.subtract, op1=mybir.AluOpType.mult,
                    )
                    nc.vector.tensor_mul(y, y, gamma_t)
                    nc.vector.tensor_add(out=y, in0=y, in1=beta_t)
                eng = nc.sync if idx % 2 == 0 else nc.gpsimd
                eng.dma_start(out[row0:row0 + 128, :], y)

            # ---------------- preamble compute
            with nc.named_scope("preamble"):
                kproj_unit(0)
                vproj_unit(0, 0)
                qproj_unit(0, 0)
                kproj_unit(1)

            # ---------------- explicit filler schedule (sub-unit granularity)
            # consumed one per ktp slot in order; leftovers emitted at unit end
            K = lambda jt, kc: (lambda: ksub(jt, kc))
            Q = lambda qc, jt: (lambda: qproj_unit(qc, jt))
            V = lambda rc, jc, rt: (lambda: vsub(rc, jc, rt))
            FILL = {}
            FILL[(0, 0)] = ([lambda rc=rc: vproj_unit(rc, 0) for rc in range(1, 8)]
                            + [Q(0, 1)])
            FILL[(0, 1)] = [Q(0, 2), V(0, 1, 0), V(0, 1, 1), V(1, 1, 0), V(1, 1, 1),
                            K(2, 0), K(2, 1), K(2, 2), K(2, 3)]
            FILL[(0, 2)] = [Q(0, 3), V(2, 1, 0), V(2, 1, 1), V(3, 1, 0), V(3, 1, 1),
                            K(3, 0), K(3, 1), K(3, 2), K(3, 3)]
            FILL[(0, 3)] = [Q(0, 4), V(4, 1, 0), V(4, 1, 1), V(5, 1, 0), V(5, 1, 1),
                            K(4, 0), K(4, 1), K(4, 2), K(4, 3)]
            FILL[(0, 4)] = [Q(0, 5), V(6, 1, 0), V(6, 1, 1), V(7, 1, 0), V(7, 1, 1),
                            K(5, 0), K(5, 1), K(5, 2), K(5, 3)]
            FILL[(0, 5)] = [Q(0, 6), Q(0, 7), K(6, 0), K(6, 1), K(6, 2), K(6, 3),
                            lambda: qstage_dma(1)]
            FILL[(0, 6)] = [Q(1, 0), Q(1, 1), Q(1, 2), K(7, 0), K(7, 1), K(7, 2),
                            K(7, 3), Q(1, 3)]
            FILL[(0, 7)] = [Q(1, 4), Q(1, 5), Q(1, 6), Q(1, 7), wo_dma]
            FILL[(1, 0)] = [lambda: qn_dma(0, 0),
                            lambda: osub(0, 0, 0, 0, 4), lambda: osub(0, 0, 0, 4, 8),
                            lambda: osub(0, 0, 1, 0, 4), lambda: osub(0, 0, 1, 4, 8),
                            lambda: obn(0, 0)]
            FILL[(1, 1)] = [lambda: qn_dma(0, 1),
                            lambda: osub(0, 1, 0, 0, 4), lambda: osub(0, 1, 0, 4, 8),
                            lambda: osub(0, 1, 1, 0, 4), lambda: osub(0, 1, 1, 4, 8),
                            lambda: obn(0, 1)]
            FILL[(1, 2)] = [lambda: qn_dma(0, 2),
                            lambda: osub(0, 2, 0, 0, 4), lambda: osub(0, 2, 0, 4, 8),
                            lambda: osub(0, 2, 1, 0, 4), lambda: osub(0, 2, 1, 4, 8),
                            lambda: obn(0, 2)]
            FILL[(1, 3)] = [lambda: qn_dma(0, 3),
                            lambda: osub(0, 3, 0, 0, 4), lambda: osub(0, 3, 0, 4, 8),
                            lambda: osub(0, 3, 1, 0, 4), lambda: osub(0, 3, 1, 4, 8),
                            lambda: obn(0, 3)]

            # gate for LN(qc0) sqrt ops: ready only once all four mv are done
            eps3_t = qp.tile([128, 1], FP32, tag="eps3", bufs=1)

            def ln0_gate():
                nc.vector.tensor_scalar(
                    out=eps3_t, in0=mv_t[3][:, 0:1], scalar1=0.0, scalar2=1e-5,
                    op0=mybir.AluOpType.mult, op1=mybir.AluOpType.add,
                )

            FILL[(1, 4)] = [ln0_gate, lambda: sqrt_batch(0, 4, eps3_t),
                            lambda: ln_finish(0), lambda: ln_finish(1),
                            lambda: ln_finish(2), lambda: ln_finish(3)]
            FILL[(1, 5)] = []
            FILL[(1, 6)] = [lambda: qn_dma(1, 0), lambda: qn_dma(1, 1),
                            lambda: qn_dma(1, 2),
                            lambda: osub(1, 0, 0, 0, 6), lambda: osub(1, 0, 1, 0, 6),
                            lambda: osub(1, 1, 0, 0, 6), lambda: osub(1, 1, 1, 0, 6),
                            lambda: qn_dma(1, 3)]
            FILL[(1, 7)] = [lambda: osub(1, 2, 0, 0, 6), lambda: osub(1, 2, 1, 0, 6),
                            lambda: osub(1, 3, 0, 0, 6), lambda: osub(1, 3, 1, 0, 6)]

            # ---------------- attention
            units = [(qc, hp) for qc in range(2) for hp in range(8)]
            states = {}

            def emit_sc(u, kt):
                qc_, hp_ = u
                st = states[u]
                sc = ps.tile([128, 1024], FP32, tag="sc", bufs=2)
                nc.tensor.matmul(
                    sc[:, 0:512],
                    ksT[hp_][0:64, kt * 128:(kt + 1) * 128],
                    qs_t[qc_][hp_][0:64, :], start=True, stop=True,
                    skip_group_check=True,
                )
                nc.tensor.matmul(
                    sc[:, 512:1024],
                    ksT[hp_][64:128, kt * 128:(kt + 1) * 128],
                    qs_t[qc_][hp_][64:128, :], start=True, stop=True,
                    tile_position=(64, 0), skip_group_check=True,
                )
                if kt % 2 == 0:
                    E = qp.tile([128, 2, 2, 512], F8E5, tag="E", bufs=2)
                    st[kt // 2] = E
                else:
                    E = st[kt // 2]
                i = kt % 2
                nc.scalar.activation(
                    E[:, i:i + 1, :, :], sc, func=EXP,
                    bias=neg2_t[:, :], scale=0.125,
                )

            def prelude(u):
                states[u] = {}
                emit_sc(u, 0)
                emit_sc(u, 1)

            pending = [None]
            prelude(units[0])
            for ui, u in enumerate(units):
                qc, hp = u
                fills = list(FILL[u])
                with nc.named_scope(f"attn{qc}"):
                    ue = ps.tile([128, 512], FP32, tag="ue")
                    uo = ps.tile([128, 512], FP32, tag="uo")
                    st = states[u]
                    for ktp in range(8):
                        if 2 * ktp + 2 < 16:
                            emit_sc(u, 2 * ktp + 2)
                        if 2 * ktp + 3 < 16:
                            emit_sc(u, 2 * ktp + 3)
                        if ktp == 3 and pending[0] is not None:
                            emit_norm(pending[0])
                            pending[0] = None
                        if fills:
                            fills.pop(0)()
                        if ktp == 7 and ui + 1 < len(units):
                            prelude(units[ui + 1])
                        Ep = st.pop(ktp)
                        nc.tensor.matmul(
                            ue[0:65, :],
                            vsp[ktp][:, :, 2 * hp:2 * hp + 1, :],
                            Ep[:, :, 0:1, :],
                            start=(ktp == 0), stop=(ktp == 7),
                            perf_mode=DR, skip_group_check=True,
                        )
                        nc.tensor.matmul(
                            uo[0:65, :],
                            vsp[ktp][:, :, 2 * hp + 1:2 * hp + 2, :],
                            Ep[:, :, 1:2, :],
                            start=(ktp == 0), stop=(ktp == 7),
                            perf_mode=DR, skip_group_check=True,
                        )
                    while fills:
                        fills.pop(0)()
                    ues = qp.tile([128, 512], BF16, tag="ues", bufs=2)
                    uos = qp.tile([128, 512], BF16, tag="uos", bufs=2)
                    nc.vector.tensor_copy(ues[0:65, :], ue[0:65, :])
                    nc.vector.tensor_copy(uos[0:65, :], uo[0:65, :])
                    pending[0] = (ues, uos, hp, qc)
                    if hp == 7:
                        emit_norm(pending[0])
                        pending[0] = None

            # ---------------- tail: O-proj qc1 it6..7 + LN of qc1
            with nc.named_scope("tail"):
                for rt in range(4):
                    osub(1, rt, 0, 6, 8)
                    osub(1, rt, 1, 6, 8)
                    obn(1, rt)
                sqrt_batch(4, 8, eps_t)
                for idx in range(4, 8):
                    ln_finish(idx)
    nc.finalize()
    return nc


def _pack_w(W, dt):
    return np.ascontiguousarray(
        np.asarray(W).T.reshape(8, 128, D).transpose(1, 0, 2)).astype(dt)


def _pack_inputs(qb, kb, vb, wqh, wkh, wvh, woh, g, bt, qnb):
    kS = np.ascontiguousarray(
        np.asarray(kb).T.reshape(8, 128, 4, 512).transpose(1, 2, 0, 3)).astype(f8)
    vS = np.ascontiguousarray(
        np.asarray(vb).T.reshape(8, 128, 8, 256).transpose(1, 2, 0, 3)).astype(f8)
    qS = np.ascontiguousarray(
        np.asarray(qb).T.reshape(8, 128, 2, 512).transpose(1, 2, 0, 3)).astype(f8)
    return {
        "kS": kS, "vS": vS, "qS": qS,
        "wkp": wkh, "wvp": wvh, "wqp": wqh, "wop": woh,
        "qn": qnb, "gamma": g, "beta": bt,
    }


def kernel(q, k, v, Wq, Wk, Wv, Wo, gamma, beta, _trace=False):
    global _NC, LAST_EXEC_NS, LAST_RESULT
    if _NC is None:
        fg = bool(np.all(np.asarray(gamma) == 1.0))
        _NC = _build(fuse_gamma=fg)
    wqh = _pack_w(Wq, f8)
    wkh = _pack_w(Wk, f8)
    wvh = _pack_w(Wv, f8)
    woh = _pack_w(Wo, bf16)
    g = np.ascontiguousarray(np.asarray(gamma, dtype=np.float32).reshape(1, D))
    bt = np.ascontiguousarray(np.asarray(beta, dtype=np.float32).reshape(1, D))
    in_maps = []
    for c in range(8):
        b, hh = divmod(c, 2)
        qb = q[b, hh * NQ:(hh + 1) * NQ, :]
        in_maps.append(_pack_inputs(
            qb, k[b], v[b], wqh, wkh, wvh, woh, g, bt,
            np.ascontiguousarray(qb, dtype=np.float32),
        ))
    res = bass_utils.run_bass_kernel_spmd(_NC, in_maps, list(range(8)), trace=_trace)
    LAST_EXEC_NS = getattr(res, "exec_time_ns", None)
    LAST_RESULT = res
    outp = np.empty((B, N, D), np.float32)
    for c in range(8):
        b, hh = divmod(c, 2)
        outp[b, hh * NQ:(hh + 1) * NQ, :] = res.results[c]["out"]
    return outp


# revision 23
# speedup vs baseline: 1.4447x; 1.0061x over previous
import numpy as np
import ml_dtypes

import concourse.bacc as bacc
import concourse.bass as bass
import concourse.mybir as mybir
import concourse.tile as tile
from concourse import bass_utils

bf16 = ml_dtypes.bfloat16
f8 = ml_dtypes.float8_e4m3

B, N, D = 4, 2048, 1024
NQ, NK = 1024, 2048
FP32 = mybir.dt.float32
BF16 = mybir.dt.bfloat16
F8 = mybir.dt.float8e4
F8E5 = mybir.dt.float8e5
EXP = mybir.ActivationFunctionType.Exp
SQRT = mybir.ActivationFunctionType.Sqrt
DR = mybir.MatmulPerfMode.DoubleRow

LAST_EXEC_NS = None
LAST_RESULT = None
_NC = None


def _broadcast_ap(dram_ap, parts):
    return bass.AP(
        tensor=dram_ap.tensor,
        offset=dram_ap.offset,
        ap=[[0, parts], dram_ap.ap[-1]],
    )


def _build(fuse_gamma=True):
    nc = bacc.Bacc(None, target_bir_lowering=False)
    # host-packed SBUF-image inputs (contiguous per partition)
    kS = nc.dram_tensor("kS", [128, 4, 8, 512], F8, kind="ExternalInput")
    vS = nc.dram_tensor("vS", [128, 8, 8, 256], F8, kind="ExternalInput")
    qS = nc.dram_tensor("qS", [128, 2, 8, 512], F8, kind="ExternalInput")
    wkp = nc.dram_tensor("wkp", [128, 8, D], F8, kind="ExternalInput")
    wvp = nc.dram_tensor("wvp", [128, 8, D], F8, kind="ExternalInput")
    wqp = nc.dram_tensor("wqp", [128, 8, D], F8, kind="ExternalInput")
    wop = nc.dram_tensor("wop", [128, 8, D], BF16, kind="ExternalInput")
    qn = nc.dram_tensor("qn", [NQ, D], FP32, kind="ExternalInput")
    gamma = nc.dram_tensor("gamma", [1, D], FP32, kind="ExternalInput")
    beta = nc.dram_tensor("beta", [1, D], FP32, kind="ExternalInput")
    out = nc.dram_tensor("out", [NQ, D], FP32, kind="ExternalOutput")

    with tile.TileContext(nc) as tc:
        with (
            tc.tile_pool(name="perm", bufs=1) as perm,
            tc.tile_pool(name="qp", bufs=1) as qp,
            tc.tile_pool(name="ps", bufs=1, space="PSUM") as ps,
        ):
            beta_t = perm.tile([128, D], FP32)
            nc.gpsimd.dma_start(out=beta_t, in_=_broadcast_ap(beta[0:1, :], 128))
            if not fuse_gamma:
                gamma_t = perm.tile([128, D], FP32)
                nc.gpsimd.dma_start(out=gamma_t, in_=_broadcast_ap(gamma[0:1, :], 128))
            eps_t = perm.tile([128, 1], FP32)
            nc.vector.memset(eps_t, 1e-5)
            ones_f = perm.tile([128, 64], BF16)
            nc.vector.memset(ones_f, 1.0)
            neg2_t = perm.tile([128, 1], FP32)
            nc.vector.memset(neg2_t, -2.0)

            wk_t = perm.tile([128, 8, D], F8, name="wk_t")
            wv_t = perm.tile([128, 8, D], F8, name="wv_t")
            wq_t = perm.tile([128, 8, D], F8, name="wq_t")
            wo_t = perm.tile([128, 8, D], BF16, name="wo_t")
            nc.sync.dma_start(wk_t, wkp[:, :, :])
            nc.gpsimd.dma_start(wv_t, wvp[:, :, :])
            nc.gpsimd.dma_start(wq_t, wqp[:, :, :])

            kstage = perm.tile([128, 4, 8, 512], F8, name="kstage")
            nc.scalar.dma_start(kstage[:, 0:2, :, :], kS[:, 0:2, :, :])
            nc.sync.dma_start(kstage[:, 2:4, :, :], kS[:, 2:4, :, :])
            qstage = perm.tile([128, 8, 512], F8, name="qstage")
            nc.sync.dma_start(qstage, qS[:, 0, :, :])

            # persistent activation tensors
            ksT = [perm.tile([128, NK], BF16, name=f"ks{j}") for j in range(8)]
            vsp = [perm.tile([128, 2, 16, 65], F8, name=f"vsp{t}") for t in range(8)]
            qs_t = [
                [perm.tile([128, 512], BF16, name=f"qs{c}_{j}") for j in range(8)]
                for c in range(2)
            ]
            at_t = [
                [perm.tile([128, 512], BF16, name=f"at{c}_{j}") for j in range(8)]
                for c in range(2)
            ]
            outf_t = [perm.tile([128, D], FP32, name=f"outf{r}") for r in range(8)]
            mv_t = [perm.tile([128, 2], FP32, name=f"mv{r}") for r in range(8)]

            def ksub(jt, kc):
                pp = ps.tile([128, 512], FP32, tag="pp", bufs=2)
                for t in range(4):
                    nc.tensor.matmul(
                        pp,
                        wk_t[:, 2 * t:2 * t + 2, jt * 128:(jt + 1) * 128],
                        kstage[:, kc, 2 * t:2 * t + 2, :],
                        start=(t == 0), stop=(t == 3),
                        perf_mode=DR, skip_group_check=True,
                    )
                nc.vector.tensor_copy(ksT[jt][:, kc * 512:(kc + 1) * 512], pp)

            def kproj_unit(jt):
                for kc in range(4):
                    ksub(jt, kc)

            def qstage_dma(qc):
                nc.sync.dma_start(qstage, qS[:, qc, :, :])

            def qproj_unit(qc, jt):
                pp = ps.tile([128, 512], FP32, tag="pp", bufs=2)
                for t in range(4):
                    nc.tensor.matmul(
                        pp,
                        wq_t[:, 2 * t:2 * t + 2, jt * 128:(jt + 1) * 128],
                        qstage[:, 2 * t:2 * t + 2, :],
                        start=(t == 0), stop=(t == 3),
                        perf_mode=DR, skip_group_check=True,
                    )
                nc.vector.tensor_copy(qs_t[qc][jt], pp)

            vstages = {}

            def vsub(rc, jc, rt):
                if rt == 0:
                    vstages[(rc, jc)] = qp.tile(
                        [128, 8, 256], F8, tag="vstage", bufs=2, name=f"vst{rc}_{jc}"
                    )
                    nc.gpsimd.dma_start(vstages[(rc, jc)], vS[:, rc, :, :])
                vstage = vstages[(rc, jc)]
                pp = ps.tile([128, 512], FP32, tag="pp", bufs=2)
                for t in range(4):
                    nc.tensor.matmul(
                        pp,
                        vstage[:, 2 * t:2 * t + 2, rt * 128:(rt + 1) * 128],
                        wv_t[:, 2 * t:2 * t + 2, jc * 512:(jc + 1) * 512],
                        start=(t == 0), stop=(t == 3),
                        perf_mode=DR, skip_group_check=True,
                    )
                nc.vector.tensor_copy(
                    vsp[rc][:, rt:rt + 1, jc * 8:(jc + 1) * 8, 0:64],
                    pp.rearrange("p (h c) -> p h c", h=8),
                )
                nc.vector.memset(vsp[rc][:, rt:rt + 1, jc * 8:(jc + 1) * 8, 64:65], 1.0)

            def vproj_unit(rc, jc):
                vsub(rc, jc, 0)
                vsub(rc, jc, 1)

            def wo_dma():
                nc.sync.dma_start(wo_t, wop[:, :, :])

            def emit_norm(p):
                ues_, uos_, hp_, qc_ = p
                bce = ps.tile([128, 512], FP32, tag="pp", bufs=2)
                bco = ps.tile([128, 512], FP32, tag="pp", bufs=2)
                nc.tensor.matmul(
                    bce[0:64, :], ones_f[64:65, 0:64], ues_[64:65, :],
                    start=True, stop=True, tile_position=(64, 0),
                    skip_group_check=True,
                )
                nc.tensor.matmul(
                    bco[0:64, :], ones_f[64:65, 0:64], uos_[64:65, :],
                    start=True, stop=True, tile_position=(64, 0),
                    skip_group_check=True,
                )
                br1 = qp.tile([128, 512], FP32, tag="rd", bufs=2)
                br2 = qp.tile([128, 512], FP32, tag="rd", bufs=2)
                nc.vector.reciprocal_approx_fast(br1[0:64, :], bce[0:64, :])
                nc.vector.reciprocal_approx_fast(br2[0:64, :], bco[0:64, :])
                nc.vector.tensor_tensor(
                    at_t[qc_][hp_][0:64, :], ues_[0:64, :], br1[0:64, :],
                    mybir.AluOpType.mult,
                )
                nc.vector.tensor_tensor(
                    at_t[qc_][hp_][64:128, :], uos_[0:64, :], br2[0:64, :],
                    mybir.AluOpType.mult,
                )

            def qn_dma(qc, rt):
                idx = qc * 4 + rt
                row0 = qc * 512 + rt * 128
                nc.sync.dma_start(outf_t[idx], qn[row0:row0 + 128, :])

            def osub(qc, rt, oc, it0, it1):
                idx = qc * 4 + rt
                outf = outf_t[idx]
                po = ps.tile([128, 512], FP32, tag="pp", bufs=2)
                for it in range(it0, it1):
                    nc.tensor.matmul(
                        po, at_t[qc][it][:, rt * 128:(rt + 1) * 128],
                        wo_t[:, it, oc * 512:(oc + 1) * 512],
                        start=(it == it0), stop=(it == it1 - 1),
                        skip_group_check=True,
                    )
                nc.vector.tensor_add(
                    out=outf[:, oc * 512:(oc + 1) * 512],
                    in0=outf[:, oc * 512:(oc + 1) * 512], in1=po,
                )

            def obn(qc, rt):
                idx = qc * 4 + rt
                outf = outf_t[idx]
                bst = qp.tile([128, 2, 6], FP32, tag="bst", bufs=2)
                for sg in range(2):
                    nc.vector.bn_stats(
                        out=bst[:, sg, :], in_=outf[:, sg * 512:(sg + 1) * 512]
                    )
                nc.vector.bn_aggr(out=mv_t[idx], in_=bst)

            def oproj_unit(qc, rt):
                qn_dma(qc, rt)
                osub(qc, rt, 0, 0, 8)
                osub(qc, rt, 1, 0, 8)
                obn(qc, rt)

            def sqrt_batch(lo, hi, bias_t):
                for idx in range(lo, hi):
                    nc.scalar.activation(
                        out=mv_t[idx][:, 1:2], in_=mv_t[idx][:, 1:2],
                        func=SQRT, bias=bias_t[:, :], scale=1.0,
                    )

            def ln_finish(idx):
                row0 = (idx // 4) * 512 + (idx % 4) * 128
                mv = mv_t[idx]
                outf = outf_t[idx]
                nc.vector.reciprocal(mv[:, 1:2], mv[:, 1:2])
                y = qp.tile([128, D], FP32, tag="y", bufs=2)
                if fuse_gamma:
                    nmu = qp.tile([128, 1], FP32, tag="nmu", bufs=2)
                    nc.vector.tensor_scalar(
                        out=nmu, in0=mv[:, 0:1], scalar1=mv[:, 1:2], scalar2=-1.0,
                        op0=mybir.AluOpType.mult, op1=mybir.AluOpType.mult,
                    )
                    nc.vector.affine_then_add(
                        y, outf, beta_t, scale=mv[:, 1:2], bias=nmu[:, 0:1],
                    )
                else:
                    nc.vector.tensor_scalar(
                        out=y, in0=outf, scalar1=mv[:, 0:1], scalar2=mv[:, 1:2],
                        op0=mybir.AluOpType.subtract, op1=mybir.AluOpType.mult,
                    )
                    nc.vector.tensor_mul(y, y, gamma_t)
                    nc.vector.tensor_add(out=y, in0=y, in1=beta_t)
                eng = nc.sync if idx % 2 == 0 else nc.gpsimd
                eng.dma_start(out[row0:row0 + 128, :], y)

            # ---------------- preamble compute
            with nc.named_scope("preamble"):
                kproj_unit(0)
                vproj_unit(0, 0)
                qproj_unit(0, 0)
                kproj_unit(1)

            # ---------------- explicit filler schedule (sub-unit granularity)
            # consumed one per ktp slot in order; leftovers emitted at unit end
            K = lambda jt, kc: (lambda: ksub(jt, kc))
            Q = lambda qc, jt: (lambda: qproj_unit(qc, jt))
            V = lambda rc, jc, rt: (lambda: vsub(rc, jc, rt))
            FILL = {}
            FILL[(0, 0)] = ([lambda rc=rc: vproj_unit(rc, 0) for rc in range(1, 8)]
                            + [Q(0, 1)])
            FILL[(0, 1)] = [Q(0, 2), V(0, 1, 0), V(0, 1, 1), V(1, 1, 0), V(1, 1, 1),
                            K(2, 0), K(2, 1), K(2, 2), K(2, 3)]
            FILL[(0, 2)] = [Q(0, 3), V(2, 1, 0), V(2, 1, 1), V(3, 1, 0), V(3, 1, 1),
                            K(3, 0), K(3, 1), K(3, 2), K(3, 3)]
            FILL[(0, 3)] = [Q(0, 4), V(4, 1, 0), V(4, 1, 1), V(5, 1, 0), V(5, 1, 1),
                            K(4, 0), K(4, 1), K(4, 2), K(4, 3)]
            FILL[(0, 4)] = [Q(0, 5), V(6, 1, 0), V(6, 1, 1), V(7, 1, 0), V(7, 1, 1),
                            K(5, 0), K(5, 1), K(5, 2), K(5, 3)]
            FILL[(0, 5)] = [Q(0, 6), Q(0, 7), K(6, 0), K(6, 1), K(6, 2), K(6, 3),
                            lambda: qstage_dma(1)]
            FILL[(0, 6)] = [Q(1, 0), Q(1, 1), Q(1, 2), K(7, 0), K(7, 1), K(7, 2),
                            K(7, 3), Q(1, 3)]
            FILL[(0, 7)] = [Q(1, 4), Q(1, 5), Q(1, 6), Q(1, 7), wo_dma]
            FILL[(1, 0)] = [lambda: qn_dma(0, 0),
                            lambda: osub(0, 0, 0, 0, 4), lambda: osub(0, 0, 0, 4, 8),
                            lambda: osub(0, 0, 1, 0, 4), lambda: osub(0, 0, 1, 4, 8),
                            lambda: obn(0, 0)]
            FILL[(1, 1)] = [lambda: qn_dma(0, 1),
                            lambda: osub(0, 1, 0, 0, 4), lambda: osub(0, 1, 0, 4, 8),
                            lambda: osub(0, 1, 1, 0, 4), lambda: osub(0, 1, 1, 4, 8),
                            lambda: obn(0, 1)]
            FILL[(1, 2)] = [lambda: qn_dma(0, 2),
                            lambda: osub(0, 2, 0, 0, 4), lambda: osub(0, 2, 0, 4, 8),
                            lambda: osub(0, 2, 1, 0, 4), lambda: osub(0, 2, 1, 4, 8),
                            lambda: obn(0, 2)]
            FILL[(1, 3)] = [lambda: qn_dma(0, 3),
                            lambda: osub(0, 3, 0, 0, 4), lambda: osub(0, 3, 0, 4, 8),
                            lambda: osub(0, 3, 1, 0, 4), lambda: osub(0, 3, 1, 4, 8),
                            lambda: obn(0, 3)]

            # gate for LN(qc0) sqrt ops: ready only once all four mv are done
            eps3_t = qp.tile([128, 1], FP32, tag="eps3", bufs=1)

            def ln0_gate():
                nc.vector.tensor_scalar(
                    out=eps3_t, in0=mv_t[3][:, 0:1], scalar1=0.0, scalar2=1e-5,
                    op0=mybir.AluOpType.mult, op1=mybir.AluOpType.add,
                )

            FILL[(1, 4)] = [ln0_gate, lambda: sqrt_batch(0, 4, eps3_t),
                            lambda: ln_finish(0), lambda: ln_finish(1),
                            lambda: ln_finish(2), lambda: ln_finish(3)]
            FILL[(1, 5)] = []
            FILL[(1, 6)] = [lambda: qn_dma(1, 0), lambda: qn_dma(1, 1),
                            lambda: qn_dma(1, 2),
                            lambda: osub(1, 0, 0, 0, 6), lambda: osub(1, 0, 1, 0, 6),
                            lambda: osub(1, 1, 0, 0, 6), lambda: osub(1, 1, 1, 0, 6),
                            lambda: qn_dma(1, 3)]
            FILL[(1, 7)] = [lambda: osub(1, 2, 0, 0, 6), lambda: osub(1, 2, 1, 0, 6),
                            lambda: osub(1, 3, 0, 0, 6), lambda: osub(1, 3, 1, 0, 6)]

            # ---------------- attention
            units = [(qc, hp) for qc in range(2) for hp in range(8)]
            states = {}

            def emit_sc(u, kt):
                qc_, hp_ = u
                st = states[u]
                sc = ps.tile([128, 1024], FP32, tag="sc", bufs=2)
                nc.tensor.matmul(
                    sc[:, 0:512],
                    ksT[hp_][0:64, kt * 128:(kt + 1) * 128],
                    qs_t[qc_][hp_][0:64, :], start=True, stop=True,
                    skip_group_check=True,
                )
                nc.tensor.matmul(
                    sc[:, 512:1024],
                    ksT[hp_][64:128, kt * 128:(kt + 1) * 128],
                    qs_t[qc_][hp_][64:128, :], start=True, stop=True,
                    tile_position=(64, 0), skip_group_check=True,
                )
                if kt % 2 == 0:
                    E = qp.tile([128, 2, 2, 512], F8E5, tag="E", bufs=2)
                    st[kt // 2] = E
                else:
                    E = st[kt // 2]
                i = kt % 2
                nc.scalar.activation(
                    E[:, i:i + 1, :, :], sc, func=EXP,
                    bias=neg2_t[:, :], scale=0.125,
                )

            def prelude(u):
                states[u] = {}
                emit_sc(u, 0)
                emit_sc(u, 1)

            pending = [None]
            prelude(units[0])
            for ui, u in enumerate(units):
                qc, hp = u
                fills = list(FILL[u])
                with nc.named_scope(f"attn{qc}"):
                    ue = ps.tile([128, 512], FP32, tag="ue")
                    uo = ps.tile([128, 512], FP32, tag="uo")
                    st = states[u]
                    for ktp in range(8):
                        if 2 * ktp + 2 < 16:
                            emit_sc(u, 2 * ktp + 2)
                        if 2 * ktp + 3 < 16:
                            emit_sc(u, 2 * ktp + 3)
                        if ktp == 3 and pending[0] is not None:
                            emit_norm(pending[0])
                            pending[0] = None
                        if fills:
                            fills.pop(0)()
                        if ktp == 7 and ui + 1 < len(units):
                            prelude(units[ui + 1])
                        Ep = st.pop(ktp)
                        nc.tensor.matmul(
                            ue[0:65, :],
                            vsp[ktp][:, :, 2 * hp:2 * hp + 1, :],
                            Ep[:, :, 0:1, :],
                            start=(ktp == 0), stop=(ktp == 7),
                            perf_mode=DR, skip_group_check=True,
                        )
                        nc.tensor.matmul(
                            uo[0:65, :],
                            vsp[ktp][:, :, 2 * hp + 1:2 * hp + 2, :],
                            Ep[:, :, 1:2, :],
                            start=(ktp == 0), stop=(ktp == 7),
                            perf_mode=DR, skip_group_check=True,
                        )
                    while fills:
                        fills.pop(0)()
                    ues = qp.tile([128, 512], BF16, tag="ues", bufs=2)
                    uos = qp.tile([128, 512], BF16, tag="uos", bufs=2)
                    nc.vector.tensor_copy(ues[0:65, :], ue[0:65, :])
                    nc.vector.tensor_copy(uos[0:65, :], uo[0:65, :])
                    pending[0] = (ues, uos, hp, qc)
                    if hp == 7:
                        emit_norm(pending[0])
                        pending[0] = None

            # ---------------- tail: O-proj qc1 it6..7 + LN of qc1
            with nc.named_scope("tail"):
                for rt in range(4):
                    osub(1, rt, 0, 6, 8)
                    osub(1, rt, 1, 6, 8)
                    obn(1, rt)
                    sqrt_batch(4 + rt, 5 + rt, eps_t)
                    ln_finish(4 + rt)
    nc.finalize()
    return nc


def _pack_w(W, dt):
    return np.ascontiguousarray(
        np.asarray(W).T.reshape(8, 128, D).transpose(1, 0, 2)).astype(dt)


def _pack_inputs(qb, kb, vb, wqh, wkh, wvh, woh, g, bt, qnb):
    kS = np.ascontiguousarray(
        np.asarray(kb).T.reshape(8, 128, 4, 512).transpose(1, 2, 0, 3)).astype(f8)
    vS = np.ascontiguousarray(
        np.asarray(vb).T.reshape(8, 128, 8, 256).transpose(1, 2, 0, 3)).astype(f8)
    qS = np.ascontiguousarray(
        np.asarray(qb).T.reshape(8, 128, 2, 512).transpose(1, 2, 0, 3)).astype(f8)
    return {
        "kS": kS, "vS": vS, "qS": qS,
        "wkp": wkh, "wvp": wvh, "wqp": wqh, "wop": woh,
        "qn": qnb, "gamma": g, "beta": bt,
    }


def kernel(q, k, v, Wq, Wk, Wv, Wo, gamma, beta, _trace=False):
    global _NC, LAST_EXEC_NS, LAST_RESULT
    if _NC is None:
        fg = bool(np.all(np.asarray(gamma) == 1.0))
        _NC = _build(fuse_gamma=fg)
    wqh = _pack_w(Wq, f8)
    wkh = _pack_w(Wk, f8)
    wvh = _pack_w(Wv, f8)
    woh = _pack_w(Wo, bf16)
    g = np.ascontiguousarray(np.asarray(gamma, dtype=np.float32).reshape(1, D))
    bt = np.ascontiguousarray(np.asarray(beta, dtype=np.float32).reshape(1, D))
    in_maps = []
    for c in range(8):
        b, hh = divmod(c, 2)
        qb = q[b, hh * NQ:(hh + 1) * NQ, :]
        in_maps.append(_pack_inputs(
            qb, k[b], v[b], wqh, wkh, wvh, woh, g, bt,
            np.ascontiguousarray(qb, dtype=np.float32),
        ))
    res = bass_utils.run_bass_kernel_spmd(_NC, in_maps, list(range(8)), trace=_trace)
    LAST_EXEC_NS = getattr(res, "exec_time_ns", None)
    LAST_RESULT = res
    outp = np.empty((B, N, D), np.float32)
    for c in range(8):
        b, hh = divmod(c, 2)
        outp[b, hh * NQ:(hh + 1) * NQ, :] = res.results[c]["out"]
    return outp


# revision 24
# speedup vs baseline: 1.4516x; 1.0048x over previous
import numpy as np
import ml_dtypes

import concourse.bacc as bacc
import concourse.bass as bass
import concourse.mybir as mybir
import concourse.tile as tile
from concourse import bass_utils

bf16 = ml_dtypes.bfloat16
f8 = ml_dtypes.float8_e4m3

B, N, D = 4, 2048, 1024
NQ, NK = 1024, 2048
FP32 = mybir.dt.float32
BF16 = mybir.dt.bfloat16
F8 = mybir.dt.float8e4
F8E5 = mybir.dt.float8e5
EXP = mybir.ActivationFunctionType.Exp
SQRT = mybir.ActivationFunctionType.Sqrt
DR = mybir.MatmulPerfMode.DoubleRow

LAST_EXEC_NS = None
LAST_RESULT = None
_NC = None


def _broadcast_ap(dram_ap, parts):
    return bass.AP(
        tensor=dram_ap.tensor,
        offset=dram_ap.offset,
        ap=[[0, parts], dram_ap.ap[-1]],
    )


def _build(fuse_gamma=True):
    nc = bacc.Bacc(None, target_bir_lowering=False)
    # host-packed SBUF-image inputs (contiguous per partition)
    kS = nc.dram_tensor("kS", [128, 4, 8, 512], F8, kind="ExternalInput")
    vS = nc.dram_tensor("vS", [128, 8, 8, 256], F8, kind="ExternalInput")
    qS = nc.dram_tensor("qS", [128, 2, 8, 512], F8, kind="ExternalInput")
    wkp = nc.dram_tensor("wkp", [128, 8, D], F8, kind="ExternalInput")
    wvp = nc.dram_tensor("wvp", [128, 8, D], F8, kind="ExternalInput")
    wqp = nc.dram_tensor("wqp", [128, 8, D], F8, kind="ExternalInput")
    wop = nc.dram_tensor("wop", [128, 8, D], BF16, kind="ExternalInput")
    qn = nc.dram_tensor("qn", [NQ, D], FP32, kind="ExternalInput")
    gamma = nc.dram_tensor("gamma", [1, D], FP32, kind="ExternalInput")
    beta = nc.dram_tensor("beta", [1, D], FP32, kind="ExternalInput")
    out = nc.dram_tensor("out", [NQ, D], FP32, kind="ExternalOutput")

    with tile.TileContext(nc) as tc:
        with (
            tc.tile_pool(name="perm", bufs=1) as perm,
            tc.tile_pool(name="qp", bufs=1) as qp,
            tc.tile_pool(name="ps", bufs=1, space="PSUM") as ps,
        ):
            beta_t = perm.tile([128, D], FP32)
            nc.gpsimd.dma_start(out=beta_t, in_=_broadcast_ap(beta[0:1, :], 128))
            if not fuse_gamma:
                gamma_t = perm.tile([128, D], FP32)
                nc.gpsimd.dma_start(out=gamma_t, in_=_broadcast_ap(gamma[0:1, :], 128))
            eps_t = perm.tile([128, 1], FP32)
            nc.vector.memset(eps_t, 1e-5)
            ones_f = perm.tile([128, 64], BF16)
            nc.vector.memset(ones_f, 1.0)
            neg2_t = perm.tile([128, 1], FP32)
            nc.vector.memset(neg2_t, -2.0)

            wk_t = perm.tile([128, 8, D], F8, name="wk_t")
            wv_t = perm.tile([128, 8, D], F8, name="wv_t")
            wq_t = perm.tile([128, 8, D], F8, name="wq_t")
            wo_t = perm.tile([128, 8, D], BF16, name="wo_t")
            nc.sync.dma_start(wk_t, wkp[:, :, :])
            nc.gpsimd.dma_start(wq_t, wqp[:, :, :])

            kstage = perm.tile([128, 4, 8, 512], F8, name="kstage")
            nc.scalar.dma_start(kstage[:, 0:2, :, :], kS[:, 0:2, :, :])
            nc.scalar.dma_start(wv_t, wvp[:, :, :])
            nc.sync.dma_start(kstage[:, 2:4, :, :], kS[:, 2:4, :, :])
            qstage = perm.tile([128, 8, 512], F8, name="qstage")
            nc.sync.dma_start(qstage, qS[:, 0, :, :])

            # persistent activation tensors
            ksT = [perm.tile([128, NK], BF16, name=f"ks{j}") for j in range(8)]
            vsp = [perm.tile([128, 2, 16, 65], F8, name=f"vsp{t}") for t in range(8)]
            qs_t = [
                [perm.tile([128, 512], BF16, name=f"qs{c}_{j}") for j in range(8)]
                for c in range(2)
            ]
            at_t = [
                [perm.tile([128, 512], BF16, name=f"at{c}_{j}") for j in range(8)]
                for c in range(2)
            ]
            outf_t = [perm.tile([128, D], FP32, name=f"outf{r}") for r in range(8)]
            mv_t = [perm.tile([128, 2], FP32, name=f"mv{r}") for r in range(8)]

            def ksub(jt, kc):
                pp = ps.tile([128, 512], FP32, tag="pp", bufs=2)
                for t in range(4):
                    nc.tensor.matmul(
                        pp,
                        wk_t[:, 2 * t:2 * t + 2, jt * 128:(jt + 1) * 128],
                        kstage[:, kc, 2 * t:2 * t + 2, :],
                        start=(t == 0), stop=(t == 3),
                        perf_mode=DR, skip_group_check=True,
                    )
                nc.vector.tensor_copy(ksT[jt][:, kc * 512:(kc + 1) * 512], pp)

            def kproj_unit(jt):
                for kc in range(4):
                    ksub(jt, kc)

            def qstage_dma(qc):
                nc.sync.dma_start(qstage, qS[:, qc, :, :])

            def qproj_unit(qc, jt):
                pp = ps.tile([128, 512], FP32, tag="pp", bufs=2)
                for t in range(4):
                    nc.tensor.matmul(
                        pp,
                        wq_t[:, 2 * t:2 * t + 2, jt * 128:(jt + 1) * 128],
                        qstage[:, 2 * t:2 * t + 2, :],
                        start=(t == 0), stop=(t == 3),
                        perf_mode=DR, skip_group_check=True,
                    )
                nc.vector.tensor_copy(qs_t[qc][jt], pp)

            vstages = {}

            def vsub(rc, jc, rt):
                if rt == 0:
                    vstages[(rc, jc)] = qp.tile(
                        [128, 8, 256], F8, tag="vstage", bufs=2, name=f"vst{rc}_{jc}"
                    )
                    nc.gpsimd.dma_start(vstages[(rc, jc)], vS[:, rc, :, :])
                vstage = vstages[(rc, jc)]
                pp = ps.tile([128, 512], FP32, tag="pp", bufs=2)
                for t in range(4):
                    nc.tensor.matmul(
                        pp,
                        vstage[:, 2 * t:2 * t + 2, rt * 128:(rt + 1) * 128],
                        wv_t[:, 2 * t:2 * t + 2, jc * 512:(jc + 1) * 512],
                        start=(t == 0), stop=(t == 3),
                        perf_mode=DR, skip_group_check=True,
                    )
                nc.vector.tensor_copy(
                    vsp[rc][:, rt:rt + 1, jc * 8:(jc + 1) * 8, 0:64],
                    pp.rearrange("p (h c) -> p h c", h=8),
                )
                nc.vector.memset(vsp[rc][:, rt:rt + 1, jc * 8:(jc + 1) * 8, 64:65], 1.0)

            def vproj_unit(rc, jc):
                vsub(rc, jc, 0)
                vsub(rc, jc, 1)

            def wo_dma():
                nc.sync.dma_start(wo_t, wop[:, :, :])

            def emit_norm(p):
                ues_, uos_, hp_, qc_ = p
                bce = ps.tile([128, 512], FP32, tag="pp", bufs=2)
                bco = ps.tile([128, 512], FP32, tag="pp", bufs=2)
                nc.tensor.matmul(
                    bce[0:64, :], ones_f[64:65, 0:64], ues_[64:65, :],
                    start=True, stop=True, tile_position=(64, 0),
                    skip_group_check=True,
                )
                nc.tensor.matmul(
                    bco[0:64, :], ones_f[64:65, 0:64], uos_[64:65, :],
                    start=True, stop=True, tile_position=(64, 0),
                    skip_group_check=True,
                )
                br1 = qp.tile([128, 512], FP32, tag="rd", bufs=2)
                br2 = qp.tile([128, 512], FP32, tag="rd", bufs=2)
                nc.vector.reciprocal_approx_fast(br1[0:64, :], bce[0:64, :])
                nc.vector.reciprocal_approx_fast(br2[0:64, :], bco[0:64, :])
                nc.vector.tensor_tensor(
                    at_t[qc_][hp_][0:64, :], ues_[0:64, :], br1[0:64, :],
                    mybir.AluOpType.mult,
                )
                nc.vector.tensor_tensor(
                    at_t[qc_][hp_][64:128, :], uos_[0:64, :], br2[0:64, :],
                    mybir.AluOpType.mult,
                )

            def qn_dma(qc, rt):
                idx = qc * 4 + rt
                row0 = qc * 512 + rt * 128
                nc.sync.dma_start(outf_t[idx], qn[row0:row0 + 128, :])

            def osub(qc, rt, oc, it0, it1):
                idx = qc * 4 + rt
                outf = outf_t[idx]
                po = ps.tile([128, 512], FP32, tag="pp", bufs=2)
                for it in range(it0, it1):
                    nc.tensor.matmul(
                        po, at_t[qc][it][:, rt * 128:(rt + 1) * 128],
                        wo_t[:, it, oc * 512:(oc + 1) * 512],
                        start=(it == it0), stop=(it == it1 - 1),
                        skip_group_check=True,
                    )
                nc.vector.tensor_add(
                    out=outf[:, oc * 512:(oc + 1) * 512],
                    in0=outf[:, oc * 512:(oc + 1) * 512], in1=po,
                )

            def obn(qc, rt):
                idx = qc * 4 + rt
                outf = outf_t[idx]
                bst = qp.tile([128, 2, 6], FP32, tag="bst", bufs=2)
                for sg in range(2):
                    nc.vector.bn_stats(
                        out=bst[:, sg, :], in_=outf[:, sg * 512:(sg + 1) * 512]
                    )
                nc.vector.bn_aggr(out=mv_t[idx], in_=bst)

            def oproj_unit(qc, rt):
                qn_dma(qc, rt)
                osub(qc, rt, 0, 0, 8)
                osub(qc, rt, 1, 0, 8)
                obn(qc, rt)

            def sqrt_batch(lo, hi, bias_t):
                for idx in range(lo, hi):
                    nc.scalar.activation(
                        out=mv_t[idx][:, 1:2], in_=mv_t[idx][:, 1:2],
                        func=SQRT, bias=bias_t[:, :], scale=1.0,
                    )

            def ln_finish(idx):
                row0 = (idx // 4) * 512 + (idx % 4) * 128
                mv = mv_t[idx]
                outf = outf_t[idx]
                nc.vector.reciprocal(mv[:, 1:2], mv[:, 1:2])
                y = qp.tile([128, D], FP32, tag="y", bufs=2)
                if fuse_gamma:
                    nmu = qp.tile([128, 1], FP32, tag="nmu", bufs=2)
                    nc.vector.tensor_scalar(
                        out=nmu, in0=mv[:, 0:1], scalar1=mv[:, 1:2], scalar2=-1.0,
                        op0=mybir.AluOpType.mult, op1=mybir.AluOpType.mult,
                    )
                    nc.vector.affine_then_add(
                        y, outf, beta_t, scale=mv[:, 1:2], bias=nmu[:, 0:1],
                    )
                else:
                    nc.vector.tensor_scalar(
                        out=y, in0=outf, scalar1=mv[:, 0:1], scalar2=mv[:, 1:2],
                        op0=mybir.AluOpType.subtract, op1=mybir.AluOpType.mult,
                    )
                    nc.vector.tensor_mul(y, y, gamma_t)
                    nc.vector.tensor_add(out=y, in0=y, in1=beta_t)
                nc.sync.dma_start(out[row0:row0 + 64, :], y[0:64, :])
                nc.gpsimd.dma_start(out[row0 + 64:row0 + 128, :], y[64:128, :])

            # ---------------- preamble compute
            with nc.named_scope("preamble"):
                kproj_unit(0)
                vproj_unit(0, 0)
                qproj_unit(0, 0)
                kproj_unit(1)

            # ---------------- explicit filler schedule (sub-unit granularity)
            # consumed one per ktp slot in order; leftovers emitted at unit end
            K = lambda jt, kc: (lambda: ksub(jt, kc))
            Q = lambda qc, jt: (lambda: qproj_unit(qc, jt))
            V = lambda rc, jc, rt: (lambda: vsub(rc, jc, rt))
            FILL = {}
            FILL[(0, 0)] = ([lambda rc=rc: vproj_unit(rc, 0) for rc in range(1, 8)]
                            + [Q(0, 1)])
            FILL[(0, 1)] = [Q(0, 2), V(0, 1, 0), V(0, 1, 1), V(1, 1, 0), V(1, 1, 1),
                            K(2, 0), K(2, 1), K(2, 2), K(2, 3)]
            FILL[(0, 2)] = [Q(0, 3), V(2, 1, 0), V(2, 1, 1), V(3, 1, 0), V(3, 1, 1),
                            K(3, 0), K(3, 1), K(3, 2), K(3, 3)]
            FILL[(0, 3)] = [Q(0, 4), V(4, 1, 0), V(4, 1, 1), V(5, 1, 0), V(5, 1, 1),
                            K(4, 0), K(4, 1), K(4, 2), K(4, 3)]
            FILL[(0, 4)] = [Q(0, 5), V(6, 1, 0), V(6, 1, 1), V(7, 1, 0), V(7, 1, 1),
                            K(5, 0), K(5, 1), K(5, 2), K(5, 3)]
            FILL[(0, 5)] = [Q(0, 6), Q(0, 7), K(6, 0), K(6, 1), K(6, 2), K(6, 3),
                            lambda: qstage_dma(1)]
            FILL[(0, 6)] = [Q(1, 0), Q(1, 1), Q(1, 2), K(7, 0), K(7, 1), K(7, 2),
                            K(7, 3), Q(1, 3)]
            FILL[(0, 7)] = [Q(1, 4), Q(1, 5), Q(1, 6), Q(1, 7), wo_dma]
            FILL[(1, 0)] = [lambda: qn_dma(0, 0),
                            lambda: osub(0, 0, 0, 0, 4), lambda: osub(0, 0, 0, 4, 8),
                            lambda: osub(0, 0, 1, 0, 4), lambda: osub(0, 0, 1, 4, 8),
                            lambda: obn(0, 0)]
            FILL[(1, 1)] = [lambda: qn_dma(0, 1),
                            lambda: osub(0, 1, 0, 0, 4), lambda: osub(0, 1, 0, 4, 8),
                            lambda: osub(0, 1, 1, 0, 4), lambda: osub(0, 1, 1, 4, 8),
                            lambda: obn(0, 1)]
            FILL[(1, 2)] = [lambda: qn_dma(0, 2),
                            lambda: osub(0, 2, 0, 0, 4), lambda: osub(0, 2, 0, 4, 8),
                            lambda: osub(0, 2, 1, 0, 4), lambda: osub(0, 2, 1, 4, 8),
                            lambda: obn(0, 2)]
            FILL[(1, 3)] = [lambda: qn_dma(0, 3),
                            lambda: osub(0, 3, 0, 0, 4), lambda: osub(0, 3, 0, 4, 8),
                            lambda: osub(0, 3, 1, 0, 4), lambda: osub(0, 3, 1, 4, 8),
                            lambda: obn(0, 3)]

            # gate for LN(qc0) sqrt ops: ready only once all four mv are done
            eps3_t = qp.tile([128, 1], FP32, tag="eps3", bufs=1)

            def ln0_gate():
                nc.vector.tensor_scalar(
                    out=eps3_t, in0=mv_t[3][:, 0:1], scalar1=0.0, scalar2=1e-5,
                    op0=mybir.AluOpType.mult, op1=mybir.AluOpType.add,
                )

            FILL[(1, 4)] = [ln0_gate, lambda: sqrt_batch(0, 4, eps3_t),
                            lambda: ln_finish(0), lambda: ln_finish(1),
                            lambda: ln_finish(2), lambda: ln_finish(3)]
            FILL[(1, 5)] = []
            FILL[(1, 6)] = [lambda: qn_dma(1, 0), lambda: qn_dma(1, 1),
                            lambda: qn_dma(1, 2),
                            lambda: osub(1, 0, 0, 0, 6), lambda: osub(1, 0, 1, 0, 6),
                            lambda: osub(1, 1, 0, 0, 6), lambda: osub(1, 1, 1, 0, 6),
                            lambda: qn_dma(1, 3)]
            FILL[(1, 7)] = [lambda: osub(1, 2, 0, 0, 6), lambda: osub(1, 2, 1, 0, 6),
                            lambda: osub(1, 3, 0, 0, 6), lambda: osub(1, 3, 1, 0, 6)]

            # ---------------- attention
            units = [(qc, hp) for qc in range(2) for hp in range(8)]
            states = {}

            def emit_sc(u, kt):
                qc_, hp_ = u
                st = states[u]
                sc = ps.tile([128, 1024], FP32, tag="sc", bufs=2)
                nc.tensor.matmul(
                    sc[:, 0:512],
                    ksT[hp_][0:64, kt * 128:(kt + 1) * 128],
                    qs_t[qc_][hp_][0:64, :], start=True, stop=True,
                    skip_group_check=True,
                )
                nc.tensor.matmul(
                    sc[:, 512:1024],
                    ksT[hp_][64:128, kt * 128:(kt + 1) * 128],
                    qs_t[qc_][hp_][64:128, :], start=True, stop=True,
                    tile_position=(64, 0), skip_group_check=True,
                )
                if kt % 2 == 0:
                    E = qp.tile([128, 2, 2, 512], F8E5, tag="E", bufs=2)
                    st[kt // 2] = E
                else:
                    E = st[kt // 2]
                i = kt % 2
                nc.scalar.activation(
                    E[:, i:i + 1, :, :], sc, func=EXP,
                    bias=neg2_t[:, :], scale=0.125,
                )

            def prelude(u):
                states[u] = {}
                emit_sc(u, 0)
                emit_sc(u, 1)

            pending = [None]
            prelude(units[0])
            for ui, u in enumerate(units):
                qc, hp = u
                fills = list(FILL[u])
                with nc.named_scope(f"attn{qc}"):
                    ue = ps.tile([128, 512], FP32, tag="ue")
                    uo = ps.tile([128, 512], FP32, tag="uo")
                    st = states[u]
                    for ktp in range(8):
                        if 2 * ktp + 2 < 16:
                            emit_sc(u, 2 * ktp + 2)
                        if 2 * ktp + 3 < 16:
                            emit_sc(u, 2 * ktp + 3)
                        if ktp == 3 and pending[0] is not None:
                            emit_norm(pending[0])
                            pending[0] = None
                        if fills:
                            fills.pop(0)()
                        if ktp == 7 and ui + 1 < len(units):
                            prelude(units[ui + 1])
                        Ep = st.pop(ktp)
                        nc.tensor.matmul(
                            ue[0:65, :],
                            vsp[ktp][:, :, 2 * hp:2 * hp + 1, :],
                            Ep[:, :, 0:1, :],
                            start=(ktp == 0), stop=(ktp == 7),
                            perf_mode=DR, skip_group_check=True,
                        )
                        nc.tensor.matmul(
                            uo[0:65, :],
                            vsp[ktp][:, :, 2 * hp + 1:2 * hp + 2, :],
                            Ep[:, :, 1:2, :],
                            start=(ktp == 0), stop=(ktp == 7),
                            perf_mode=DR, skip_group_check=True,
                        )
                    while fills:
                        fills.pop(0)()
                    ues = qp.tile([128, 512], BF16, tag="ues", bufs=2)
                    uos = qp.tile([128, 512], BF16, tag="uos", bufs=2)
                    nc.vector.tensor_copy(ues[0:65, :], ue[0:65, :])
                    nc.vector.tensor_copy(uos[0:65, :], uo[0:65, :])
                    pending[0] = (ues, uos, hp, qc)
                    if hp == 7:
                        emit_norm(pending[0])
                        pending[0] = None

            # ---------------- tail: O-proj qc1 it6..7 + LN of qc1
            with nc.named_scope("tail"):
                for rt in range(4):
                    osub(1, rt, 0, 6, 8)
                    osub(1, rt, 1, 6, 8)
                    obn(1, rt)
                    sqrt_batch(4 + rt, 5 + rt, eps_t)
                    ln_finish(4 + rt)
    nc.finalize()
    return nc


def _pack_w(W, dt):
    return np.ascontiguousarray(
        np.asarray(W).T.reshape(8, 128, D).transpose(1, 0, 2)).astype(dt)


def _pack_inputs(qb, kb, vb, wqh, wkh, wvh, woh, g, bt, qnb):
    kS = np.ascontiguousarray(
        np.asarray(kb).T.reshape(8, 128, 4, 512).transpose(1, 2, 0, 3)).astype(f8)
    vS = np.ascontiguousarray(
        np.asarray(vb).T.reshape(8, 128, 8, 256).transpose(1, 2, 0, 3)).astype(f8)
    qS = np.ascontiguousarray(
        np.asarray(qb).T.reshape(8, 128, 2, 512).transpose(1, 2, 0, 3)).astype(f8)
    return {
        "kS": kS, "vS": vS, "qS": qS,
        "wkp": wkh, "wvp": wvh, "wqp": wqh, "wop": woh,
        "qn": qnb, "gamma": g, "beta": bt,
    }


def kernel(q, k, v, Wq, Wk, Wv, Wo, gamma, beta, _trace=False):
    global _NC, LAST_EXEC_NS, LAST_RESULT
    if _NC is None:
        fg = bool(np.all(np.asarray(gamma) == 1.0))
        _NC = _build(fuse_gamma=fg)
    wqh = _pack_w(Wq, f8)
    wkh = _pack_w(Wk, f8)
    wvh = _pack_w(Wv, f8)
    woh = _pack_w(Wo, bf16)
    g = np.ascontiguousarray(np.asarray(gamma, dtype=np.float32).reshape(1, D))
    bt = np.ascontiguousarray(np.asarray(beta, dtype=np.float32).reshape(1, D))
    in_maps = []
    for c in range(8):
        b, hh = divmod(c, 2)
        qb = q[b, hh * NQ:(hh + 1) * NQ, :]
        in_maps.append(_pack_inputs(
            qb, k[b], v[b], wqh, wkh, wvh, woh, g, bt,
            np.ascontiguousarray(qb, dtype=np.float32),
        ))
    res = bass_utils.run_bass_kernel_spmd(_NC, in_maps, list(range(8)), trace=_trace)
    LAST_EXEC_NS = getattr(res, "exec_time_ns", None)
    LAST_RESULT = res
    outp = np.empty((B, N, D), np.float32)
    for c in range(8):
        b, hh = divmod(c, 2)
        outp[b, hh * NQ:(hh + 1) * NQ, :] = res.results[c]["out"]
    return outp
